# revision 22
# baseline (speedup 1.0000x reference)
"""AI4DEM DEM-stencil kernel for one TRN2 chip (8 NeuronCores, SPMD), fp16.

v5 = v4 (fp16 pipeline, scaled jitter positions, class trims) + Newton-3rd-law
pairing: for each stencil offset pair {s, -s} with |sz| <= 1, the pair force
field is computed once on a region extended to R (union) R+s, and accumulated
twice: once at the center cell (weight +I) and once at the neighbour
(z/x-shifted view, y-shift and sign folded into the PE matmul weight -P_sy /
+P_sy). Collision+damping are exactly antisymmetric so the second side is
free; friction re-uses the pair geometry and only recomputes the
mask-asymmetric tangential part (22 DVE ops instead of 67).

Scales: positions sigma=0.3/D (jitter-only, fp16, sentinel 30.0 in wrapped
halo cells); velocities x64; mask x64D; fn /16. Host undoes per channel.
ch0-7 accumulate on PE into PSUM (fp32), ch8-11 via SWDGE DMA-accumulate
into fp16 SBUF accumulators (partition-split in two for wrapped y-shifts).
Validated vs the fp32 reference: global rel l2 ~7.7e-3.
"""
import math
from contextlib import ExitStack

import numpy as np

import concourse.tile_sem_assignment as _tsa
_tsa.NUM_HWDGE_SEMS = 3
_tsa.NUM_SWDGE_GLOBAL_SEMS = 3
from concourse import bacc, mybir, tile
from concourse.bass_utils import run_bass_kernel_spmd

F32 = np.float32
D = 0.003
KN = 10000.0
_alpha = -math.log(0.79) / math.pi
_gamma = _alpha / math.sqrt(_alpha ** 2 + 1.0)
_mass = 4.0 / 3.0 * 3.1415926 * D ** 3 * 674.0
ETA = 2.0 * _gamma * math.sqrt(KN * _mass / 2.0)
MU = 0.43

SIG = 0.3
VS = 64.0
FN16 = 16.0
# Wrapped-halo jitter sentinels. Three distinct values so that a pair of
# DIFFERENT wrap classes (z-halo plane x y-wrap row x x-halo col) can never
# produce dj ~= 0 (fake contact); any two classes differ by >= 16 and every
# class is >= 7 away from real jitter. Values kept small so products stay
# finite in fp16 (w2 <= ~2.2e4).
SENT_Z = 8.0
SENT_Y = -8.0
SENT_X = 24.0

C_LT = float(F32((2 * SIG) ** 2))
FNI_A, FNI_B = -60.0, 100.0               # fncol*inv = 100 - 60*inv
FNP_A = float(F32(-100.0 * MU / FN16))    # fnp  = MU*|fncol|/16  (>=0 in contact)
FNP_B = float(F32(60.0 * MU / FN16))
FNN_A = float(F32(100.0 * MU / FN16))     # fnpn = -fnp
FNN_B = float(F32(-60.0 * MU / FN16))
IVT_BIAS = float(F32(VS * VS * 1e-8))

GRID = 128
NCORES = 8
ZLOC = GRID // NCORES
ZH = ZLOC + 4
XW = GRID + 4
ZC = 4
ZE, XE = ZC + 1, GRID + 2   # max extended pair region (|sz|<=1, |sx|<=2)

FIELDS = ["jx", "jy", "jz", "vx", "vy", "vz", "wx", "wy", "wz"]
ROTS = [-1, 0, 1, 2]
MROTS = [0, 1, 2]
ALL_OFFSETS = [(k - 2, j - 2, i - 2) for i in range(5) for j in range(5) for k in range(5)]
FULL_CLASSES = {(0, 0, 1), (0, 1, 1), (1, 1, 1), (0, 0, 2)}
COLDAMP_CLASSES = {(0, 1, 2)}

DT = mybir.dt.float16
DT32 = mybir.dt.float32
A = mybir.AluOpType
AF = mybir.ActivationFunctionType


def _classify(s):
    return tuple(sorted(abs(v) for v in s))


def _plan():
    """Returns (paired, unpaired): paired = list of (rep, is_full); rep has
    sy in {0,1,2}, |sz| <= 1. unpaired = list of (s, is_full) emitted a-side
    only (the |sz|=2 offsets)."""
    paired, unpaired, seen = [], [], set()
    for s in ALL_OFFSETS:
        if s == (0, 0, 0) or s in seen:
            continue
        cl = _classify(s)
        if cl in FULL_CLASSES:
            is_full = True
        elif cl in COLDAMP_CLASSES:
            is_full = False
        else:
            continue
        neg = (-s[0], -s[1], -s[2])
        if abs(s[0]) == 2:
            unpaired.append((s, is_full))
            unpaired.append((neg, is_full))
        else:
            rep = s if (s[1] > 0 or (s[1] == 0 and (s[0] > 0 or (s[0] == 0 and s[2] > 0)))) else neg
            paired.append((rep, is_full))
        seen.add(s)
        seen.add(neg)
    return paired, unpaired


DBL_TAGS = {"tmp0", "tmp1", "tmp2", "ffx", "ffy", "ffz", "tq", "stage",
            "q1", "q2", "px", "py", "pz", "djx", "djy", "djz",
            "dx", "dy", "dz", "dvx", "dvy", "dvz", "cm", "u"}


def build_kernel(temp_bufs=1, const_inside=True, use_pairs=True,
                 bside_mm=True, bside_fric=True):
    nc = bacc.Bacc("TRN2", target_bir_lowering=False, debug=False, num_devices=NCORES)

    def reg_const(value):
        key = (mybir.dt.float32, value)
        if key in nc.const_aps.aps:
            return
        t = nc.alloc_sbuf_tensor(f"const-f32-{value}", [128, 1], mybir.dt.float32)
        nc.gpsimd.memset(t.ap(), value)
        nc.const_aps.aps[key] = t.ap()

    if not const_inside:
        reg_const(0.0)
        reg_const(IVT_BIAS)

    ins = {}
    for f in FIELDS:
        for sy in ROTS:
            ins[(f, sy)] = nc.dram_tensor(
                f"{f}_r{sy + 1}", [GRID, ZH, XW], DT, kind="ExternalInput").ap()
    masks = {}
    for sy in MROTS:
        masks[sy] = nc.dram_tensor(
            f"mask_r{sy}", [GRID, ZH, XW], DT, kind="ExternalInput").ap()
    wdefs = {
        "I": None, "nI": None, "nP1": None, "nP2": None, "P1": None, "P2": None}
    for wname in list(wdefs):
        wdefs[wname] = nc.dram_tensor(
            f"w_{wname}", [GRID, GRID], DT, kind="ExternalInput").ap()
    out = nc.dram_tensor("out", [GRID, 12, ZLOC, GRID], DT, kind="ExternalOutput").ap()
    # b-side ch8-11 contributions for y-shifted pairs, accumulated unshifted;
    # the host applies the y-roll (partition-shifted SWDGE accumulates are
    # fatal on HW at scale).
    outb = nc.dram_tensor("outb", [GRID, 8, ZLOC, GRID], DT, kind="ExternalOutput").ap()
    BSY = (1, 2)

    paired, unpaired = _plan()
    if not use_pairs:
        unpaired = [(s, f) for s, f in unpaired] + \
            [(ss, f) for s, f in paired for ss in (s, (-s[0], -s[1], -s[2]))]
        paired = []
    n_a = len(paired) + len(unpaired)                      # a-side contributions
    n_b05 = len(paired)                                    # b-side ch0-5
    nfull_a = sum(1 for _, f in paired if f) + sum(1 for _, f in unpaired if f)
    nfull_b = sum(1 for _, f in paired if f)

    with tile.TileContext(nc) as tc:
        with ExitStack() as ctx:
            if const_inside:
                reg_const(0.0)
                reg_const(IVT_BIAS)
            cpool = ctx.enter_context(tc.tile_pool(name="center", bufs=1))
            spool = ctx.enter_context(tc.tile_pool(name="shift", bufs=1))
            apool = ctx.enter_context(tc.tile_pool(name="accum", bufs=1))
            tpool = ctx.enter_context(tc.tile_pool(name="temps", bufs=temp_bufs))
            ppool = ctx.enter_context(tc.tile_pool(name="psum", bufs=1, space="PSUM"))

            wt = {}
            for wname, drt in wdefs.items():
                t = cpool.tile([GRID, GRID], DT, tag=f"w_{wname}", name=f"w_{wname}")
                nc.sync.dma_start(t[:], drt[:, :])
                wt[wname] = t
            WB = {0: ("nI", "I"), 1: ("nP1", "P1"), 2: ("nP2", "P2")}

            fdh = (ZC + 4) * XW

            for c0 in range(0, ZLOC, ZC):
                ctiles = {}
                for f in FIELDS:
                    t = cpool.tile([GRID, fdh], DT, tag=f"c_{f}")
                    nc.sync.dma_start(t[:], ins[(f, 0)][:, c0:c0 + ZC + 4, :])
                    ctiles[f] = t
                mtiles = {}
                for sy in MROTS:
                    t = cpool.tile([GRID, fdh], DT, tag=f"m_{sy}")
                    nc.sync.dma_start(t[:], masks[sy][:, c0:c0 + ZC + 4, :])
                    mtiles[sy] = t

                psums = {}
                for ch in range(8):
                    psums[ch] = ppool.tile([GRID, ZC * GRID], DT32, tag=f"ps{ch}",
                                           name=f"ps{ch}")
                acc16 = {}
                for ch in range(8, 12):
                    at = apool.tile([GRID, ZC * GRID], DT, tag=f"acc{ch}",
                                    name=f"acc{ch}")
                    nc.gpsimd.memset(at[:], 0.0)
                    acc16[ch] = at
                accb = {}
                for syb in BSY:
                    for ch in range(8, 12):
                        at = apool.tile([GRID, ZC * GRID], DT,
                                        tag=f"accb{ch}_{syb}",
                                        name=f"accb{ch}_{syb}")
                        nc.gpsimd.memset(at[:], 0.0)
                        accb[(ch, syb)] = at

                pe_seen = {ch: False for ch in range(8)}
                pe_done = {ch: 0 for ch in range(8)}
                n_contrib = {}
                for ch in range(6):
                    n_contrib[ch] = n_a + (n_b05 if bside_mm else 0)
                n_contrib[6] = n_contrib[7] = nfull_a + \
                    (nfull_b if (bside_mm and bside_fric) else 0)

                def pe_accum(ch, rhs, w="I"):
                    pe_done[ch] += 1
                    nc.tensor.matmul(
                        psums[ch][:], wt[w][:], rhs,
                        start=not pe_seen[ch],
                        stop=pe_done[ch] == n_contrib[ch],
                        skip_group_check=True,
                    )
                    pe_seen[ch] = True

                def pool_accum(ch, src3d, sy):
                    """sy == 0: acc16[ch] += src3d; else accb[(ch, sy)] +=
                    src3d (the host rolls it into place)."""
                    t = acc16[ch] if sy == 0 else accb[(ch, sy)]
                    dst = t[:].rearrange("p (z x) -> p z x", x=GRID)
                    nc.gpsimd.dma_start(dst, src3d, accum_op=A.add)

                def T(tag, bufs=None):
                    if bufs is None and tag in DBL_TAGS:
                        bufs = 2
                    return tpool.tile([GRID, ZE, XE], DT, tag=tag, name=tag,
                                      bufs=bufs)

                V, S = nc.vector, nc.scalar

                def emit(s, b_side):
                    """Emit offset s (a-side on R, or R u R+s when b_side),
                    plus (when b_side) the mirrored -s contributions."""
                    sz, sy, sx = s
                    full = _classify(s) in FULL_CLASSES
                    za0 = min(0, sz) if b_side else 0
                    xa0 = min(0, sx) if b_side else 0
                    zaE = ZC + abs(sz) if b_side else ZC
                    xaE = GRID + abs(sx) if b_side else GRID

                    def tv(tag, bufs=None):
                        return T(tag, bufs=bufs)[:][:, 0:zaE, 0:xaE]

                    def cv(f):
                        v = ctiles[f][:].rearrange("p (z x) -> p z x", x=XW)
                        return v[:, za0 + 2:za0 + 2 + zaE, xa0 + 2:xa0 + 2 + xaE]

                    def sv(f):
                        v = stiles[f][:].rearrange("p (z x) -> p z x", x=XW)
                        return v[:, za0 + 2 - sz:za0 + 2 - sz + zaE,
                                 xa0 + 2 - sx:xa0 + 2 - sx + xaE]

                    def mview(t, dz, dx):
                        v = t[:].rearrange("p (z x) -> p z x", x=XW)
                        return v[:, za0 + 2 + dz:za0 + 2 + dz + zaE,
                                 xa0 + 2 + dx:xa0 + 2 + dx + xaE]

                    def aview(t3):
                        return t3[:, -za0:-za0 + ZC, -xa0:-xa0 + GRID]

                    def bview(t3):
                        return t3[:, sz - za0:sz - za0 + ZC,
                                  sx - xa0:sx - xa0 + GRID]

                    # --- pair geometry -------------------------------------
                    dj = {}
                    d = {}
                    for ax, f, so in (("x", "jx", sx), ("y", "jy", sy), ("z", "jz", sz)):
                        djt = tv(f"dj{ax}")
                        V.tensor_tensor(djt, cv(f), sv(f), A.subtract)
                        dj[ax] = djt
                        if so:
                            dt_ = tv(f"d{ax}")
                            V.tensor_scalar(dt_, djt, float(F32(SIG * so)), None, A.add)
                            d[ax] = dt_
                        else:
                            d[ax] = djt
                    p = {}
                    for ax in "xyz":
                        pt = tv(f"p{ax}")
                        S.activation(pt, d[ax], AF.Square)
                        p[ax] = pt
                    r2 = tv("r2")
                    V.tensor_tensor(r2, p["x"], p["y"], A.add)
                    V.tensor_tensor(r2, r2, p["z"], A.add)
                    inv = tv("inv")
                    S.activation(inv, r2, AF.Abs_reciprocal_sqrt)
                    fni = tv("fni")
                    V.tensor_scalar(fni, inv, FNI_A, FNI_B, A.mult, A.add)
                    c = tv("c")
                    V.tensor_scalar(c, r2, C_LT, None, A.is_lt)
                    g = tv("g")
                    V.tensor_tensor(g, fni, c, A.mult)
                    for k, ax in ((0, "x"), (1, "y"), (2, "z")):
                        t3 = tv(f"tmp{k}")
                        V.tensor_tensor(t3, g, d[ax], A.mult)
                        pe_accum(k, aview(t3))
                        if b_side and bside_mm:
                            pe_accum(k, bview(t3), WB[sy][0])
                    dv = {}
                    for ax, f in (("x", "vx"), ("y", "vy"), ("z", "vz")):
                        dvt = tv(f"dv{ax}")
                        V.tensor_tensor(dvt, cv(f), sv(f), A.subtract)
                        dv[ax] = dvt
                    m1, m2 = tv("m1"), tv("m2")
                    V.tensor_tensor(m1, dv["x"], d["x"], A.mult)
                    V.tensor_tensor(m2, dv["y"], d["y"], A.mult)
                    s4 = tv("s4")
                    V.tensor_tensor(s4, m1, m2, A.add)
                    V.tensor_tensor(m1, dv["z"], d["z"], A.mult)
                    num = tv("num")
                    V.tensor_tensor(num, s4, m1, A.add)
                    ci = tv("ci")
                    V.tensor_tensor(ci, c, inv, A.mult)
                    t2p = tv("t2p")
                    V.tensor_tensor(t2p, num, inv, A.mult)
                    h = tv("h")
                    V.tensor_tensor(h, t2p, ci, A.mult)
                    for k, ax in ((3, "x"), (4, "y"), (5, "z")):
                        t3 = tv(f"tmp{k - 3}")
                        V.tensor_tensor(t3, h, d[ax], A.mult)
                        pe_accum(k, aview(t3))
                        if b_side and bside_mm:
                            pe_accum(k, bview(t3), WB[sy][0])
                    if not full:
                        return
                    # --- friction, a-side ----------------------------------
                    dist = tv("dist")
                    V.tensor_tensor(dist, r2, inv, A.mult)
                    fnp = tv("fnp")
                    V.tensor_scalar(fnp, dist, FNP_A, FNP_B, A.mult, A.add)
                    Fc = tv("Fc")
                    V.tensor_tensor(Fc, fnp, c, A.mult)
                    ma = {}
                    for ax in "xyz":
                        mt = tv(f"ma{ax}")
                        V.tensor_tensor(mt, d[ax], inv, A.mult)
                        ma[ax] = mt
                    sm = {}
                    for ax, f in (("x", "wx"), ("y", "wy"), ("z", "wz")):
                        smt = tv(f"sm{ax}")
                        V.tensor_tensor(smt, cv(f), sv(f), A.add)
                        sm[ax] = smt
                    cr1, cr2 = tv("cr1"), tv("cr2")
                    cross = {}
                    for ax, (a1, b1, a2, b2) in (
                            ("x", ("y", "z", "z", "y")),
                            ("y", ("z", "x", "x", "z")),
                            ("z", ("x", "y", "y", "x"))):
                        V.tensor_tensor(cr1, sm[a1], ma[b1], A.mult)
                        V.tensor_tensor(cr2, sm[a2], ma[b2], A.mult)
                        ct = tv(f"cx{ax}")
                        V.tensor_tensor(ct, cr1, cr2, A.subtract)
                        cross[ax] = ct
                    w2 = {}
                    for ax in "xyz":
                        ut = tv("u")
                        V.tensor_tensor(ut, r2, p[ax], A.subtract)
                        wt_ = tv(f"w2{ax}")
                        V.tensor_tensor(wt_, dv[ax], ut, A.mult)
                        w2[ax] = wt_
                    ma_c = mview(mtiles[0], 0, 0)
                    vt = {}
                    for ax in "xyz":
                        cm = tv("cm")
                        V.tensor_tensor(cm, cross[ax], ma_c, A.mult)
                        vtt = tv(f"vt{ax}")
                        V.tensor_tensor(vtt, w2[ax], cm, A.add)
                        vt[ax] = vtt

                    def fric_tail(vts, F3src, wname, acc_sy, swap_tq, is_b):
                        qs = tv("qs")
                        q1, q2 = tv("q1"), tv("q2")
                        S.activation(q1, vts["x"], AF.Square)
                        S.activation(q2, vts["y"], AF.Square)
                        V.tensor_tensor(qs, q1, q2, A.add)
                        S.activation(q1, vts["z"], AF.Square)
                        V.tensor_tensor(qs, qs, q1, A.add)
                        ivt = tv("ivt")
                        S.activation(ivt, qs, AF.Abs_reciprocal_sqrt, bias=IVT_BIAS)
                        F3 = tv("F3")
                        V.tensor_tensor(F3, F3src, ivt, A.mult)
                        bv = bview if is_b else aview
                        ff = {}
                        for k, ax in ((6, "x"), (7, "y")):
                            t3 = tv(f"ff{ax}")
                            V.tensor_tensor(t3, vts[ax], F3, A.mult)
                            if not (is_b and not bside_mm):
                                pe_accum(k, bv(t3), wname)
                            ff[ax] = t3
                        ffz = tv("ffz")
                        V.tensor_tensor(ffz, vts["z"], F3, A.mult)
                        ff["z"] = ffz
                        pool_accum(8, bv(ffz), acc_sy)
                        # torque is a pure side-chain (feeds only the SWDGE
                        # accumulators) -> run it on the otherwise-idle GPSIMD
                        G = nc.gpsimd
                        tc1, tc2 = tv("tcr1"), tv("tcr2")
                        for k, (a1, b1, a2, b2) in ((9, ("y", "z", "z", "y")),
                                                    (10, ("z", "x", "x", "z")),
                                                    (11, ("x", "y", "y", "x"))):
                            if swap_tq:
                                # (ff x ma)_k = ff_a1*ma_b1 - ff_a2*ma_b2
                                G.tensor_tensor(tc1, ff[a1], ma[b1], A.mult)
                                G.tensor_tensor(tc2, ff[a2], ma[b2], A.mult)
                            else:
                                G.tensor_tensor(tc1, ma[a1], ff[b1], A.mult)
                                G.tensor_tensor(tc2, ma[a2], ff[b2], A.mult)
                            tqt = tv("tq")
                            G.tensor_tensor(tqt, tc1, tc2, A.subtract)
                            pool_accum(k, bv(tqt), acc_sy)

                    fric_tail(vt, Fc, "I", 0, False, False)
                    if not b_side or not bside_fric:
                        return
                    # --- friction, b-side ----------------------------------
                    fnpn = tv("fnpn")
                    V.tensor_scalar(fnpn, dist, FNN_A, FNN_B, A.mult, A.add)
                    Fcn = tv("Fcn")
                    V.tensor_tensor(Fcn, fnpn, c, A.mult)
                    mb = mview(mtiles[sy], -sz, -sx)
                    ub = {}
                    for ax in "xyz":
                        cm = tv("cm")
                        V.tensor_tensor(cm, cross[ax], mb, A.mult)
                        ubt = tv(f"ub{ax}")
                        V.tensor_tensor(ubt, w2[ax], cm, A.add)
                        ub[ax] = ubt
                    fric_tail(ub, Fcn, WB[sy][1], sy, True, True)

                groups = {}
                for s, _f in paired:
                    groups.setdefault(s[1], []).append((s, True))
                for s, _f in unpaired:
                    groups.setdefault(s[1], []).append((s, False))

                for sy in (0, -1, 1, 2):
                    if sy not in groups:
                        continue
                    if sy == 0:
                        stiles = ctiles
                    else:
                        stiles = {}
                        for f in FIELDS:
                            t = spool.tile([GRID, fdh], DT, tag=f"s_{f}")
                            nc.sync.dma_start(t[:], ins[(f, sy)][:, c0:c0 + ZC + 4, :])
                            stiles[f] = t
                    for s, b_side in groups[sy]:
                        emit(s, b_side)

                for ch in range(8):
                    stage = tpool.tile([GRID, ZC * GRID], DT, tag="stage",
                                       name="stage")
                    nc.scalar.copy(stage[:], psums[ch][:])
                    nc.sync.dma_start(
                        out[:, ch, c0:c0 + ZC, :],
                        stage[:].rearrange("p (z x) -> p z x", x=GRID))
                for ch in range(8, 12):
                    nc.sync.dma_start(
                        out[:, ch, c0:c0 + ZC, :],
                        acc16[ch][:].rearrange("p (z x) -> p z x", x=GRID))
                for bi, syb in enumerate(BSY):
                    for ch in range(8, 12):
                        nc.sync.dma_start(
                            outb[:, bi * 4 + ch - 8, c0:c0 + ZC, :],
                            accb[(ch, syb)][:].rearrange(
                                "p (z x) -> p z x", x=GRID))

    nc.compile()
    return nc


def prep_inputs_for_core(inputs, core):
    z0 = core * ZLOC
    name_map = {
        "jx": "x_grid", "jy": "y_grid", "jz": "z_grid",
        "vx": "vx_grid", "vy": "vy_grid", "vz": "vz_grid",
        "wx": "angular_velocity_x", "wy": "angular_velocity_y",
        "wz": "angular_velocity_z",
    }
    idx = np.arange(GRID, dtype=np.float32)
    glob = {}
    for f, src in name_map.items():
        g = np.asarray(inputs[src], dtype=np.float32).reshape(GRID, GRID, GRID)
        if f == "jx":
            g = SIG * (g / D - idx[None, None, :])
        elif f == "jy":
            g = SIG * (g / D - idx[None, :, None])
        elif f == "jz":
            g = SIG * (g / D - idx[:, None, None])
        elif f in ("vx", "vy", "vz"):
            g = VS * g
        glob[f] = g.astype(np.float16)

    im = {}
    zidx_mod = (np.arange(z0 - 2, z0 + ZLOC + 2) % GRID)
    xidx_mod = (np.arange(-2, GRID + 2) % GRID)
    z_valid = [(k, z0 - 2 + k) for k in range(ZH) if 0 <= z0 - 2 + k < GRID]
    for f in FIELDS:
        g = glob[f]
        sentinel = f in ("jx", "jy", "jz")
        for sy in ROTS:
            if sentinel:
                arr = np.full((ZH, GRID, XW), SENT_X, np.float16)
                ys = slice(max(0, sy), GRID + min(0, sy))
                yg = slice(max(0, -sy), GRID + min(0, -sy))
                for k, zg in z_valid:
                    arr[k, ys, 2:GRID + 2] = g[zg][yg]
                    if sy > 0:
                        arr[k, 0:sy, :] = SENT_Y
                    elif sy < 0:
                        arr[k, GRID + sy:GRID, :] = SENT_Y
                valid_ks = {k for k, _ in z_valid}
                for k in range(ZH):
                    if k not in valid_ks:
                        arr[k, :, :] = SENT_Z
            else:
                yidx = (np.arange(GRID) - sy) % GRID
                arr = g[zidx_mod][:, yidx][:, :, xidx_mod]
            im[f"{f}_r{sy + 1}"] = np.ascontiguousarray(arr.transpose(1, 0, 2))
    gm = np.asarray(inputs["mask"], dtype=np.float32).reshape(GRID, GRID, GRID)
    gm = (np.float32(VS * D) * gm).astype(np.float16)
    for sy in MROTS:
        yidx = (np.arange(GRID) - sy) % GRID
        arr = gm[zidx_mod][:, yidx][:, :, xidx_mod]
        im[f"mask_r{sy}"] = np.ascontiguousarray(arr.transpose(1, 0, 2))
    eye = np.eye(GRID, dtype=np.float16)
    im["w_I"] = eye
    im["w_nI"] = -eye
    for sy in (1, 2):
        # W[k, m] = 1 where k = (m+sy) % 128  => psum[m] += rhs[(m+sy)%128]
        P = np.zeros((GRID, GRID), np.float16)
        for m in range(GRID):
            P[(m + sy) % GRID, m] = 1.0
        im[f"w_P{sy}"] = P
        im[f"w_nP{sy}"] = -P
    return im


def assemble_output(core_outs):
    full = np.zeros((12, 1, 1, GRID, GRID, GRID), np.float32)
    for m, (co, cb) in enumerate(core_outs):
        slab = co.astype(np.float32)            # [y, 12, z, x]
        cbf = cb.astype(np.float32)             # [y, 8, z, x]
        for bi, syb in enumerate((1, 2)):
            # device stored b-side ch8-11 at partition a_y; dest is a_y - sy
            slab[:, 8:12] += np.roll(cbf[:, bi * 4:(bi + 1) * 4], -syb, axis=0)
        full[:, 0, 0, m * ZLOC:(m + 1) * ZLOC] = slab.transpose(1, 2, 0, 3)
    scale = np.ones(12, np.float32)
    scale[3:6] = ETA / VS
    scale[6:9] = -FN16
    scale[9:12] = -FN16 * D
    full *= scale[:, None, None, None, None, None]
    return full


_NC_CACHE = {}


def _get_nc():
    if "nc" not in _NC_CACHE:
        _NC_CACHE["nc"] = build_kernel()
    return _NC_CACHE["nc"]


def kernel(**inputs) -> np.ndarray:
    nc = _get_nc()
    in_maps = [prep_inputs_for_core(inputs, core) for core in range(NCORES)]
    res = run_bass_kernel_spmd(nc, in_maps, core_ids=list(range(NCORES)))
    return assemble_output([(res.results[m]["out"], res.results[m]["outb"])
                            for m in range(NCORES)])


# revision 24
# speedup vs baseline: 1.0945x; 1.0945x over previous
"""AI4DEM DEM-stencil kernel for one TRN2 chip (8 NeuronCores, SPMD), fp16.

v5 = v4 (fp16 pipeline, scaled jitter positions, class trims) + Newton-3rd-law
pairing: for each stencil offset pair {s, -s} with |sz| <= 1, the pair force
field is computed once on a region extended to R (union) R+s, and accumulated
twice: once at the center cell (weight +I) and once at the neighbour
(z/x-shifted view, y-shift and sign folded into the PE matmul weight -P_sy /
+P_sy). Collision+damping are exactly antisymmetric so the second side is
free; friction re-uses the pair geometry and only recomputes the
mask-asymmetric tangential part (22 DVE ops instead of 67).

Scales: positions sigma=0.3/D (jitter-only, fp16, sentinel 30.0 in wrapped
halo cells); velocities x64; mask x64D; fn /16. Host undoes per channel.
ch0-7 accumulate on PE into PSUM (fp32), ch8-11 via SWDGE DMA-accumulate
into fp16 SBUF accumulators (partition-split in two for wrapped y-shifts).
Validated vs the fp32 reference: global rel l2 ~7.7e-3.
"""
import math
from contextlib import ExitStack

import numpy as np

import concourse.tile_sem_assignment as _tsa
_tsa.NUM_HWDGE_SEMS = 3
_tsa.NUM_SWDGE_GLOBAL_SEMS = 3
from concourse import bacc, mybir, tile
from concourse.bass_utils import run_bass_kernel_spmd

F32 = np.float32
D = 0.003
KN = 10000.0
_alpha = -math.log(0.79) / math.pi
_gamma = _alpha / math.sqrt(_alpha ** 2 + 1.0)
_mass = 4.0 / 3.0 * 3.1415926 * D ** 3 * 674.0
ETA = 2.0 * _gamma * math.sqrt(KN * _mass / 2.0)
MU = 0.43

SIG = 0.3
VS = 64.0
FN16 = 16.0
# Wrapped-halo jitter sentinels. Three distinct values so that a pair of
# DIFFERENT wrap classes (z-halo plane x y-wrap row x x-halo col) can never
# produce dj ~= 0 (fake contact); any two classes differ by >= 16 and every
# class is >= 7 away from real jitter. Values kept small so products stay
# finite in fp16 (w2 <= ~2.2e4).
SENT_Z = 8.0
SENT_Y = -8.0
SENT_X = 24.0

C_LT = float(F32((2 * SIG) ** 2))
FNI_A, FNI_B = -60.0, 100.0               # fncol*inv = 100 - 60*inv
FNP_A = float(F32(-100.0 * MU / FN16))    # fnp  = MU*|fncol|/16  (>=0 in contact)
FNP_B = float(F32(60.0 * MU / FN16))
FNN_A = float(F32(100.0 * MU / FN16))     # fnpn = -fnp
FNN_B = float(F32(-60.0 * MU / FN16))
IVT_BIAS = float(F32(VS * VS * 1e-8))

GRID = 128
NCORES = 8
ZLOC = GRID // NCORES
ZH = ZLOC + 4
XW = GRID + 4
ZC = 4
ZE, XE = ZC + 1, GRID + 2   # max extended pair region (|sz|<=1, |sx|<=2)

FIELDS = ["jx", "jy", "jz", "vx", "vy", "vz", "wx", "wy", "wz"]
ROTS = [-1, 0, 1, 2]
MROTS = [0, 1, 2]
ALL_OFFSETS = [(k - 2, j - 2, i - 2) for i in range(5) for j in range(5) for k in range(5)]
FULL_CLASSES = {(0, 0, 1), (0, 1, 1), (1, 1, 1), (0, 0, 2)}
COLDAMP_CLASSES = {(0, 1, 2)}

DT = mybir.dt.float16
DT32 = mybir.dt.float32
A = mybir.AluOpType
AF = mybir.ActivationFunctionType


def _classify(s):
    return tuple(sorted(abs(v) for v in s))


def _plan():
    """Returns (paired, unpaired): paired = list of (rep, is_full); rep has
    sy in {0,1,2}, |sz| <= 1. unpaired = list of (s, is_full) emitted a-side
    only (the |sz|=2 offsets)."""
    paired, unpaired, seen = [], [], set()
    for s in ALL_OFFSETS:
        if s == (0, 0, 0) or s in seen:
            continue
        cl = _classify(s)
        if cl in FULL_CLASSES:
            is_full = True
        elif cl in COLDAMP_CLASSES:
            is_full = False
        else:
            continue
        neg = (-s[0], -s[1], -s[2])
        if abs(s[0]) == 2:
            unpaired.append((s, is_full))
            unpaired.append((neg, is_full))
        else:
            rep = s if (s[1] > 0 or (s[1] == 0 and (s[0] > 0 or (s[0] == 0 and s[2] > 0)))) else neg
            paired.append((rep, is_full))
        seen.add(s)
        seen.add(neg)
    return paired, unpaired


DBL_TAGS = {"tmp0", "tmp1", "tmp2", "ffx", "ffy", "ffz", "tq", "stage",
            "q1", "q2", "px", "py", "pz", "djx", "djy", "djz",
            "dx", "dy", "dz", "dvx", "dvy", "dvz", "cm", "u"}


def build_kernel(temp_bufs=1, const_inside=True, use_pairs=True,
                 bside_mm=True, bside_fric=True):
    nc = bacc.Bacc("TRN2", target_bir_lowering=False, debug=False, num_devices=NCORES)

    def reg_const(value):
        key = (mybir.dt.float32, value)
        if key in nc.const_aps.aps:
            return
        t = nc.alloc_sbuf_tensor(f"const-f32-{value}", [128, 1], mybir.dt.float32)
        nc.gpsimd.memset(t.ap(), value)
        nc.const_aps.aps[key] = t.ap()

    if not const_inside:
        reg_const(0.0)
        reg_const(IVT_BIAS)

    ins = {}
    for f in FIELDS:
        for sy in ROTS:
            ins[(f, sy)] = nc.dram_tensor(
                f"{f}_r{sy + 1}", [GRID, ZH, XW], DT, kind="ExternalInput").ap()
    masks = {}
    for sy in MROTS:
        masks[sy] = nc.dram_tensor(
            f"mask_r{sy}", [GRID, ZH, XW], DT, kind="ExternalInput").ap()
    wdefs = {
        "I": None, "nI": None, "nP1": None, "nP2": None, "P1": None, "P2": None}
    for wname in list(wdefs):
        wdefs[wname] = nc.dram_tensor(
            f"w_{wname}", [GRID, GRID], DT, kind="ExternalInput").ap()
    out = nc.dram_tensor("out", [GRID, 12, ZLOC, GRID], DT, kind="ExternalOutput").ap()
    # b-side ch8-11 contributions for y-shifted pairs, accumulated unshifted;
    # the host applies the y-roll (partition-shifted SWDGE accumulates are
    # fatal on HW at scale).
    outb = nc.dram_tensor("outb", [GRID, 8, ZLOC, GRID], DT, kind="ExternalOutput").ap()
    BSY = (1, 2)

    paired, unpaired = _plan()
    if not use_pairs:
        unpaired = [(s, f) for s, f in unpaired] + \
            [(ss, f) for s, f in paired for ss in (s, (-s[0], -s[1], -s[2]))]
        paired = []
    n_a = len(paired) + len(unpaired)                      # a-side contributions
    n_b05 = len(paired)                                    # b-side ch0-5
    nfull_a = sum(1 for _, f in paired if f) + sum(1 for _, f in unpaired if f)
    nfull_b = sum(1 for _, f in paired if f)

    with tile.TileContext(nc) as tc:
        with ExitStack() as ctx:
            if const_inside:
                reg_const(0.0)
                reg_const(IVT_BIAS)
            cpool = ctx.enter_context(tc.tile_pool(name="center", bufs=1))
            spool = ctx.enter_context(tc.tile_pool(name="shift", bufs=1))
            apool = ctx.enter_context(tc.tile_pool(name="accum", bufs=1))
            tpool = ctx.enter_context(tc.tile_pool(name="temps", bufs=temp_bufs))
            ppool = ctx.enter_context(tc.tile_pool(name="psum", bufs=1, space="PSUM"))

            wt = {}
            for wname, drt in wdefs.items():
                t = cpool.tile([GRID, GRID], DT, tag=f"w_{wname}", name=f"w_{wname}")
                nc.sync.dma_start(t[:], drt[:, :])
                wt[wname] = t
            WB = {0: ("nI", "I"), 1: ("nP1", "P1"), 2: ("nP2", "P2")}

            fdh = (ZC + 4) * XW

            for c0 in range(0, ZLOC, ZC):
                ctiles = {}
                for f in FIELDS:
                    t = cpool.tile([GRID, fdh], DT, tag=f"c_{f}")
                    nc.sync.dma_start(t[:], ins[(f, 0)][:, c0:c0 + ZC + 4, :])
                    ctiles[f] = t
                mtiles = {}
                for sy in MROTS:
                    t = cpool.tile([GRID, fdh], DT, tag=f"m_{sy}")
                    nc.sync.dma_start(t[:], masks[sy][:, c0:c0 + ZC + 4, :])
                    mtiles[sy] = t

                psums = {}
                for ch in range(8):
                    psums[ch] = ppool.tile([GRID, ZC * GRID], DT32, tag=f"ps{ch}",
                                           name=f"ps{ch}")
                acc16 = {}
                for ch in range(8, 12):
                    at = apool.tile([GRID, ZC * GRID], DT, tag=f"acc{ch}",
                                    name=f"acc{ch}")
                    nc.gpsimd.memset(at[:], 0.0)
                    acc16[ch] = at
                accb = {}
                for syb in BSY:
                    for ch in range(8, 12):
                        at = apool.tile([GRID, ZC * GRID], DT,
                                        tag=f"accb{ch}_{syb}",
                                        name=f"accb{ch}_{syb}")
                        nc.gpsimd.memset(at[:], 0.0)
                        accb[(ch, syb)] = at

                pe_seen = {ch: False for ch in range(8)}
                pe_done = {ch: 0 for ch in range(8)}
                n_contrib = {}
                for ch in range(6):
                    n_contrib[ch] = n_a + (n_b05 if bside_mm else 0)
                n_contrib[6] = n_contrib[7] = nfull_a + \
                    (nfull_b if (bside_mm and bside_fric) else 0)

                def pe_accum(ch, rhs, w="I"):
                    pe_done[ch] += 1
                    nc.tensor.matmul(
                        psums[ch][:], wt[w][:], rhs,
                        start=not pe_seen[ch],
                        stop=pe_done[ch] == n_contrib[ch],
                        skip_group_check=True,
                    )
                    pe_seen[ch] = True

                def pool_accum(ch, src3d, sy):
                    """sy == 0: acc16[ch] += src3d; else accb[(ch, sy)] +=
                    src3d (the host rolls it into place)."""
                    t = acc16[ch] if sy == 0 else accb[(ch, sy)]
                    dst = t[:].rearrange("p (z x) -> p z x", x=GRID)
                    nc.gpsimd.dma_start(dst, src3d, accum_op=A.add)

                def T(tag, bufs=None):
                    if bufs is None and tag in DBL_TAGS:
                        bufs = 2
                    return tpool.tile([GRID, ZE, XE], DT, tag=tag, name=tag,
                                      bufs=bufs)

                V, S = nc.vector, nc.scalar

                def emit(s, b_side):
                    """Emit offset s (a-side on R, or R u R+s when b_side),
                    plus (when b_side) the mirrored -s contributions."""
                    sz, sy, sx = s
                    full = _classify(s) in FULL_CLASSES
                    za0 = min(0, sz) if b_side else 0
                    xa0 = min(0, sx) if b_side else 0
                    zaE = ZC + abs(sz) if b_side else ZC
                    xaE = GRID + abs(sx) if b_side else GRID

                    def tv(tag, bufs=None):
                        return T(tag, bufs=bufs)[:][:, 0:zaE, 0:xaE]

                    def cv(f):
                        v = ctiles[f][:].rearrange("p (z x) -> p z x", x=XW)
                        return v[:, za0 + 2:za0 + 2 + zaE, xa0 + 2:xa0 + 2 + xaE]

                    def sv(f):
                        v = stiles[f][:].rearrange("p (z x) -> p z x", x=XW)
                        return v[:, za0 + 2 - sz:za0 + 2 - sz + zaE,
                                 xa0 + 2 - sx:xa0 + 2 - sx + xaE]

                    def mview(t, dz, dx):
                        v = t[:].rearrange("p (z x) -> p z x", x=XW)
                        return v[:, za0 + 2 + dz:za0 + 2 + dz + zaE,
                                 xa0 + 2 + dx:xa0 + 2 + dx + xaE]

                    def aview(t3):
                        return t3[:, -za0:-za0 + ZC, -xa0:-xa0 + GRID]

                    def bview(t3):
                        return t3[:, sz - za0:sz - za0 + ZC,
                                  sx - xa0:sx - xa0 + GRID]

                    # --- pair geometry -------------------------------------
                    dj = {}
                    d = {}
                    for ax, f, so in (("x", "jx", sx), ("y", "jy", sy), ("z", "jz", sz)):
                        djt = tv(f"dj{ax}")
                        V.tensor_tensor(djt, cv(f), sv(f), A.subtract)
                        dj[ax] = djt
                        if so:
                            dt_ = tv(f"d{ax}")
                            V.tensor_scalar(dt_, djt, float(F32(SIG * so)), None, A.add)
                            d[ax] = dt_
                        else:
                            d[ax] = djt
                    p = {}
                    for ax in "xyz":
                        pt = tv(f"p{ax}")
                        S.activation(pt, d[ax], AF.Square)
                        p[ax] = pt
                    r2 = tv("r2")
                    V.tensor_tensor(r2, p["x"], p["y"], A.add)
                    V.tensor_tensor(r2, r2, p["z"], A.add)
                    inv = tv("inv")
                    S.activation(inv, r2, AF.Abs_reciprocal_sqrt)
                    fni = tv("fni")
                    V.tensor_scalar(fni, inv, FNI_A, FNI_B, A.mult, A.add)
                    c = tv("c")
                    V.tensor_scalar(c, r2, C_LT, None, A.is_lt)
                    g = tv("g")
                    V.tensor_tensor(g, fni, c, A.mult)
                    for k, ax in ((0, "x"), (1, "y"), (2, "z")):
                        t3 = tv(f"tmp{k}")
                        V.tensor_tensor(t3, g, d[ax], A.mult)
                        pe_accum(k, aview(t3))
                        if b_side and bside_mm:
                            pe_accum(k, bview(t3), WB[sy][0])
                    dv = {}
                    for ax, f in (("x", "vx"), ("y", "vy"), ("z", "vz")):
                        dvt = tv(f"dv{ax}")
                        V.tensor_tensor(dvt, cv(f), sv(f), A.subtract)
                        dv[ax] = dvt
                    m1, m2 = tv("m1"), tv("m2")
                    V.tensor_tensor(m1, dv["x"], d["x"], A.mult)
                    V.tensor_tensor(m2, dv["y"], d["y"], A.mult)
                    s4 = tv("s4")
                    V.tensor_tensor(s4, m1, m2, A.add)
                    V.tensor_tensor(m1, dv["z"], d["z"], A.mult)
                    num = tv("num")
                    V.tensor_tensor(num, s4, m1, A.add)
                    ci = tv("ci")
                    V.tensor_tensor(ci, c, inv, A.mult)
                    t2p = tv("t2p")
                    V.tensor_tensor(t2p, num, inv, A.mult)
                    h = tv("h")
                    V.tensor_tensor(h, t2p, ci, A.mult)
                    for k, ax in ((3, "x"), (4, "y"), (5, "z")):
                        t3 = tv(f"tmp{k - 3}")
                        V.tensor_tensor(t3, h, d[ax], A.mult)
                        pe_accum(k, aview(t3))
                        if b_side and bside_mm:
                            pe_accum(k, bview(t3), WB[sy][0])
                    if not full:
                        return
                    # --- friction, a-side ----------------------------------
                    dist = tv("dist")
                    V.tensor_tensor(dist, r2, inv, A.mult)
                    fnp = tv("fnp")
                    V.tensor_scalar(fnp, dist, FNP_A, FNP_B, A.mult, A.add)
                    Fc = tv("Fc")
                    V.tensor_tensor(Fc, fnp, c, A.mult)
                    ma = {}
                    for ax in "xyz":
                        mt = tv(f"ma{ax}")
                        V.tensor_tensor(mt, d[ax], inv, A.mult)
                        ma[ax] = mt
                    sm = {}
                    for ax, f in (("x", "wx"), ("y", "wy"), ("z", "wz")):
                        smt = tv(f"sm{ax}")
                        V.tensor_tensor(smt, cv(f), sv(f), A.add)
                        sm[ax] = smt
                    cr1, cr2 = tv("cr1"), tv("cr2")
                    cross = {}
                    for ax, (a1, b1, a2, b2) in (
                            ("x", ("y", "z", "z", "y")),
                            ("y", ("z", "x", "x", "z")),
                            ("z", ("x", "y", "y", "x"))):
                        V.tensor_tensor(cr1, sm[a1], ma[b1], A.mult)
                        V.tensor_tensor(cr2, sm[a2], ma[b2], A.mult)
                        ct = tv(f"cx{ax}")
                        V.tensor_tensor(ct, cr1, cr2, A.subtract)
                        cross[ax] = ct
                    w2 = {}
                    for ax in "xyz":
                        ut = tv("u")
                        V.tensor_tensor(ut, r2, p[ax], A.subtract)
                        wt_ = tv(f"w2{ax}")
                        V.tensor_tensor(wt_, dv[ax], ut, A.mult)
                        w2[ax] = wt_
                    ma_c = mview(mtiles[0], 0, 0)
                    vt = {}
                    for ax in "xyz":
                        cm = tv("cm")
                        V.tensor_tensor(cm, cross[ax], ma_c, A.mult)
                        vtt = tv(f"vt{ax}")
                        V.tensor_tensor(vtt, w2[ax], cm, A.add)
                        vt[ax] = vtt

                    def fric_tail(vts, F3src, wname, acc_sy, swap_tq, is_b):
                        qs = tv("qs")
                        q1, q2 = tv("q1"), tv("q2")
                        S.activation(q1, vts["x"], AF.Square)
                        S.activation(q2, vts["y"], AF.Square)
                        V.tensor_tensor(qs, q1, q2, A.add)
                        S.activation(q1, vts["z"], AF.Square)
                        V.tensor_tensor(qs, qs, q1, A.add)
                        ivt = tv("ivt")
                        S.activation(ivt, qs, AF.Abs_reciprocal_sqrt, bias=IVT_BIAS)
                        F3 = tv("F3")
                        V.tensor_tensor(F3, F3src, ivt, A.mult)
                        bv = bview if is_b else aview
                        ff = {}
                        for k, ax in ((6, "x"), (7, "y")):
                            t3 = tv(f"ff{ax}")
                            V.tensor_tensor(t3, vts[ax], F3, A.mult)
                            if not (is_b and not bside_mm):
                                pe_accum(k, bv(t3), wname)
                            ff[ax] = t3
                        ffz = tv("ffz")
                        V.tensor_tensor(ffz, vts["z"], F3, A.mult)
                        ff["z"] = ffz
                        pool_accum(8, bv(ffz), acc_sy)
                        for k, (a1, b1, a2, b2) in ((9, ("y", "z", "z", "y")),
                                                    (10, ("z", "x", "x", "z")),
                                                    (11, ("x", "y", "y", "x"))):
                            if swap_tq:
                                # (ff x ma)_k = ff_a1*ma_b1 - ff_a2*ma_b2
                                V.tensor_tensor(cr1, ff[a1], ma[b1], A.mult)
                                V.tensor_tensor(cr2, ff[a2], ma[b2], A.mult)
                            else:
                                V.tensor_tensor(cr1, ma[a1], ff[b1], A.mult)
                                V.tensor_tensor(cr2, ma[a2], ff[b2], A.mult)
                            tqt = tv("tq")
                            V.tensor_tensor(tqt, cr1, cr2, A.subtract)
                            pool_accum(k, bv(tqt), acc_sy)

                    fric_tail(vt, Fc, "I", 0, False, False)
                    if not b_side or not bside_fric:
                        return
                    # --- friction, b-side ----------------------------------
                    fnpn = tv("fnpn")
                    V.tensor_scalar(fnpn, dist, FNN_A, FNN_B, A.mult, A.add)
                    Fcn = tv("Fcn")
                    V.tensor_tensor(Fcn, fnpn, c, A.mult)
                    mb = mview(mtiles[sy], -sz, -sx)
                    ub = {}
                    for ax in "xyz":
                        cm = tv("cm")
                        V.tensor_tensor(cm, cross[ax], mb, A.mult)
                        ubt = tv(f"ub{ax}")
                        V.tensor_tensor(ubt, w2[ax], cm, A.add)
                        ub[ax] = ubt
                    fric_tail(ub, Fcn, WB[sy][1], sy, True, True)

                groups = {}
                for s, _f in paired:
                    groups.setdefault(s[1], []).append((s, True))
                for s, _f in unpaired:
                    groups.setdefault(s[1], []).append((s, False))

                for sy in (0, -1, 1, 2):
                    if sy not in groups:
                        continue
                    if sy == 0:
                        stiles = ctiles
                    else:
                        stiles = {}
                        for f in FIELDS:
                            t = spool.tile([GRID, fdh], DT, tag=f"s_{f}")
                            nc.sync.dma_start(t[:], ins[(f, sy)][:, c0:c0 + ZC + 4, :])
                            stiles[f] = t
                    for s, b_side in groups[sy]:
                        emit(s, b_side)

                for ch in range(8):
                    stage = tpool.tile([GRID, ZC * GRID], DT, tag="stage",
                                       name="stage")
                    nc.scalar.copy(stage[:], psums[ch][:])
                    nc.sync.dma_start(
                        out[:, ch, c0:c0 + ZC, :],
                        stage[:].rearrange("p (z x) -> p z x", x=GRID))
                for ch in range(8, 12):
                    nc.sync.dma_start(
                        out[:, ch, c0:c0 + ZC, :],
                        acc16[ch][:].rearrange("p (z x) -> p z x", x=GRID))
                for bi, syb in enumerate(BSY):
                    for ch in range(8, 12):
                        nc.sync.dma_start(
                            outb[:, bi * 4 + ch - 8, c0:c0 + ZC, :],
                            accb[(ch, syb)][:].rearrange(
                                "p (z x) -> p z x", x=GRID))

    nc.compile()
    return nc


def prep_inputs_for_core(inputs, core):
    z0 = core * ZLOC
    name_map = {
        "jx": "x_grid", "jy": "y_grid", "jz": "z_grid",
        "vx": "vx_grid", "vy": "vy_grid", "vz": "vz_grid",
        "wx": "angular_velocity_x", "wy": "angular_velocity_y",
        "wz": "angular_velocity_z",
    }
    idx = np.arange(GRID, dtype=np.float32)
    glob = {}
    for f, src in name_map.items():
        g = np.asarray(inputs[src], dtype=np.float32).reshape(GRID, GRID, GRID)
        if f == "jx":
            g = SIG * (g / D - idx[None, None, :])
        elif f == "jy":
            g = SIG * (g / D - idx[None, :, None])
        elif f == "jz":
            g = SIG * (g / D - idx[:, None, None])
        elif f in ("vx", "vy", "vz"):
            g = VS * g
        glob[f] = g.astype(np.float16)

    im = {}
    zidx_mod = (np.arange(z0 - 2, z0 + ZLOC + 2) % GRID)
    xidx_mod = (np.arange(-2, GRID + 2) % GRID)
    z_valid = [(k, z0 - 2 + k) for k in range(ZH) if 0 <= z0 - 2 + k < GRID]
    for f in FIELDS:
        g = glob[f]
        sentinel = f in ("jx", "jy", "jz")
        for sy in ROTS:
            if sentinel:
                arr = np.full((ZH, GRID, XW), SENT_X, np.float16)
                ys = slice(max(0, sy), GRID + min(0, sy))
                yg = slice(max(0, -sy), GRID + min(0, -sy))
                for k, zg in z_valid:
                    arr[k, ys, 2:GRID + 2] = g[zg][yg]
                    if sy > 0:
                        arr[k, 0:sy, :] = SENT_Y
                    elif sy < 0:
                        arr[k, GRID + sy:GRID, :] = SENT_Y
                valid_ks = {k for k, _ in z_valid}
                for k in range(ZH):
                    if k not in valid_ks:
                        arr[k, :, :] = SENT_Z
            else:
                yidx = (np.arange(GRID) - sy) % GRID
                arr = g[zidx_mod][:, yidx][:, :, xidx_mod]
            im[f"{f}_r{sy + 1}"] = np.ascontiguousarray(arr.transpose(1, 0, 2))
    gm = np.asarray(inputs["mask"], dtype=np.float32).reshape(GRID, GRID, GRID)
    gm = (np.float32(VS * D) * gm).astype(np.float16)
    for sy in MROTS:
        yidx = (np.arange(GRID) - sy) % GRID
        arr = gm[zidx_mod][:, yidx][:, :, xidx_mod]
        im[f"mask_r{sy}"] = np.ascontiguousarray(arr.transpose(1, 0, 2))
    eye = np.eye(GRID, dtype=np.float16)
    im["w_I"] = eye
    im["w_nI"] = -eye
    for sy in (1, 2):
        # W[k, m] = 1 where k = (m+sy) % 128  => psum[m] += rhs[(m+sy)%128]
        P = np.zeros((GRID, GRID), np.float16)
        for m in range(GRID):
            P[(m + sy) % GRID, m] = 1.0
        im[f"w_P{sy}"] = P
        im[f"w_nP{sy}"] = -P
    return im


def assemble_output(core_outs):
    full = np.zeros((12, 1, 1, GRID, GRID, GRID), np.float32)
    for m, (co, cb) in enumerate(core_outs):
        slab = co.astype(np.float32)            # [y, 12, z, x]
        cbf = cb.astype(np.float32)             # [y, 8, z, x]
        for bi, syb in enumerate((1, 2)):
            # device stored b-side ch8-11 at partition a_y; dest is a_y - sy
            slab[:, 8:12] += np.roll(cbf[:, bi * 4:(bi + 1) * 4], -syb, axis=0)
        full[:, 0, 0, m * ZLOC:(m + 1) * ZLOC] = slab.transpose(1, 2, 0, 3)
    scale = np.ones(12, np.float32)
    scale[3:6] = ETA / VS
    scale[6:9] = -FN16
    scale[9:12] = -FN16 * D
    full *= scale[:, None, None, None, None, None]
    return full


_NC_CACHE = {}


def _get_nc():
    if "nc" not in _NC_CACHE:
        _NC_CACHE["nc"] = build_kernel()
    return _NC_CACHE["nc"]


def kernel(**inputs) -> np.ndarray:
    nc = _get_nc()
    in_maps = [prep_inputs_for_core(inputs, core) for core in range(NCORES)]
    res = run_bass_kernel_spmd(nc, in_maps, core_ids=list(range(NCORES)))
    return assemble_output([(res.results[m]["out"], res.results[m]["outb"])
                            for m in range(NCORES)])


# revision 25
# speedup vs baseline: 1.1214x; 1.0246x over previous
"""AI4DEM DEM-stencil kernel for one TRN2 chip (8 NeuronCores, SPMD), fp16.

v5 = v4 (fp16 pipeline, scaled jitter positions, class trims) + Newton-3rd-law
pairing: for each stencil offset pair {s, -s} with |sz| <= 1, the pair force
field is computed once on a region extended to R (union) R+s, and accumulated
twice: once at the center cell (weight +I) and once at the neighbour
(z/x-shifted view, y-shift and sign folded into the PE matmul weight -P_sy /
+P_sy). Collision+damping are exactly antisymmetric so the second side is
free; friction re-uses the pair geometry and only recomputes the
mask-asymmetric tangential part (22 DVE ops instead of 67).

Scales: positions sigma=0.3/D (jitter-only, fp16, sentinel 30.0 in wrapped
halo cells); velocities x64; mask x64D; fn /16. Host undoes per channel.
ch0-7 accumulate on PE into PSUM (fp32), ch8-11 via SWDGE DMA-accumulate
into fp16 SBUF accumulators (partition-split in two for wrapped y-shifts).
Validated vs the fp32 reference: global rel l2 ~7.7e-3.
"""
import math
from contextlib import ExitStack

import numpy as np

import concourse.tile_sem_assignment as _tsa
_tsa.NUM_HWDGE_SEMS = 3
_tsa.NUM_SWDGE_GLOBAL_SEMS = 3
from concourse import bacc, mybir, tile
from concourse.bass_utils import run_bass_kernel_spmd

F32 = np.float32
D = 0.003
KN = 10000.0
_alpha = -math.log(0.79) / math.pi
_gamma = _alpha / math.sqrt(_alpha ** 2 + 1.0)
_mass = 4.0 / 3.0 * 3.1415926 * D ** 3 * 674.0
ETA = 2.0 * _gamma * math.sqrt(KN * _mass / 2.0)
MU = 0.43

SIG = 0.3
VS = 64.0
FN16 = 16.0
# Wrapped-halo jitter sentinels. Three distinct values so that a pair of
# DIFFERENT wrap classes (z-halo plane x y-wrap row x x-halo col) can never
# produce dj ~= 0 (fake contact); any two classes differ by >= 16 and every
# class is >= 7 away from real jitter. Values kept small so products stay
# finite in fp16 (w2 <= ~2.2e4).
SENT_Z = 8.0
SENT_Y = -8.0
SENT_X = 24.0

C_LT = float(F32((2 * SIG) ** 2))
FNI_A, FNI_B = -60.0, 100.0               # fncol*inv = 100 - 60*inv
FNP_A = float(F32(-100.0 * MU / FN16))    # fnp  = MU*|fncol|/16  (>=0 in contact)
FNP_B = float(F32(60.0 * MU / FN16))
FNN_A = float(F32(100.0 * MU / FN16))     # fnpn = -fnp
FNN_B = float(F32(-60.0 * MU / FN16))
IVT_BIAS = float(F32(VS * VS * 1e-8))

GRID = 128
NCORES = 8
ZLOC = GRID // NCORES
ZH = ZLOC + 4
XW = GRID + 4
ZC = 4
ZE, XE = ZC + 1, GRID + 2   # max extended pair region (|sz|<=1, |sx|<=2)

FIELDS = ["jx", "jy", "jz", "vx", "vy", "vz", "wx", "wy", "wz"]
ROTS = [-1, 0, 1, 2]
MROTS = [0, 1, 2]
ALL_OFFSETS = [(k - 2, j - 2, i - 2) for i in range(5) for j in range(5) for k in range(5)]
FULL_CLASSES = {(0, 0, 1), (0, 1, 1), (1, 1, 1), (0, 0, 2)}
COLDAMP_CLASSES = {(0, 1, 2)}

DT = mybir.dt.float16
DT32 = mybir.dt.float32
A = mybir.AluOpType
AF = mybir.ActivationFunctionType


def _classify(s):
    return tuple(sorted(abs(v) for v in s))


def _plan():
    """Returns (paired, unpaired): paired = list of (rep, is_full); rep has
    sy in {0,1,2}, |sz| <= 1. unpaired = list of (s, is_full) emitted a-side
    only (the |sz|=2 offsets)."""
    paired, unpaired, seen = [], [], set()
    for s in ALL_OFFSETS:
        if s == (0, 0, 0) or s in seen:
            continue
        cl = _classify(s)
        if cl in FULL_CLASSES:
            is_full = True
        elif cl in COLDAMP_CLASSES:
            is_full = False
        else:
            continue
        neg = (-s[0], -s[1], -s[2])
        if abs(s[0]) == 2:
            unpaired.append((s, is_full))
            unpaired.append((neg, is_full))
        else:
            rep = s if (s[1] > 0 or (s[1] == 0 and (s[0] > 0 or (s[0] == 0 and s[2] > 0)))) else neg
            paired.append((rep, is_full))
        seen.add(s)
        seen.add(neg)
    return paired, unpaired


DBL_TAGS = {"tmp0", "tmp1", "tmp2", "ffx", "ffy", "ffz", "tq", "stage",
            "q1", "q2", "px", "py", "pz", "djx", "djy", "djz",
            "dx", "dy", "dz", "dvx", "dvy", "dvz", "cm", "u",
            "r2", "inv", "c", "g", "fni", "dist", "ci", "num", "t2p", "h",
            "max", "may", "maz", "smx", "smy", "smz", "cxx", "cxy", "cxz",
            "w2x", "w2y", "w2z", "vtx", "vty", "vtz", "qs", "ivt", "Fc", "F3"}


def build_kernel(temp_bufs=1, const_inside=True, use_pairs=True,
                 bside_mm=True, bside_fric=True):
    nc = bacc.Bacc("TRN2", target_bir_lowering=False, debug=False, num_devices=NCORES)

    def reg_const(value):
        key = (mybir.dt.float32, value)
        if key in nc.const_aps.aps:
            return
        t = nc.alloc_sbuf_tensor(f"const-f32-{value}", [128, 1], mybir.dt.float32)
        nc.gpsimd.memset(t.ap(), value)
        nc.const_aps.aps[key] = t.ap()

    if not const_inside:
        reg_const(0.0)
        reg_const(IVT_BIAS)

    ins = {}
    for f in FIELDS:
        for sy in ROTS:
            ins[(f, sy)] = nc.dram_tensor(
                f"{f}_r{sy + 1}", [GRID, ZH, XW], DT, kind="ExternalInput").ap()
    masks = {}
    for sy in MROTS:
        masks[sy] = nc.dram_tensor(
            f"mask_r{sy}", [GRID, ZH, XW], DT, kind="ExternalInput").ap()
    wdefs = {
        "I": None, "nI": None, "nP1": None, "nP2": None, "P1": None, "P2": None}
    for wname in list(wdefs):
        wdefs[wname] = nc.dram_tensor(
            f"w_{wname}", [GRID, GRID], DT, kind="ExternalInput").ap()
    out = nc.dram_tensor("out", [GRID, 12, ZLOC, GRID], DT, kind="ExternalOutput").ap()
    # b-side ch8-11 contributions for y-shifted pairs, accumulated unshifted;
    # the host applies the y-roll (partition-shifted SWDGE accumulates are
    # fatal on HW at scale).
    outb = nc.dram_tensor("outb", [GRID, 8, ZLOC, GRID], DT, kind="ExternalOutput").ap()
    BSY = (1, 2)

    paired, unpaired = _plan()
    if not use_pairs:
        unpaired = [(s, f) for s, f in unpaired] + \
            [(ss, f) for s, f in paired for ss in (s, (-s[0], -s[1], -s[2]))]
        paired = []
    n_a = len(paired) + len(unpaired)                      # a-side contributions
    n_b05 = len(paired)                                    # b-side ch0-5
    nfull_a = sum(1 for _, f in paired if f) + sum(1 for _, f in unpaired if f)
    nfull_b = sum(1 for _, f in paired if f)

    with tile.TileContext(nc) as tc:
        with ExitStack() as ctx:
            if const_inside:
                reg_const(0.0)
                reg_const(IVT_BIAS)
                reg_const(100.0)
                reg_const(FNP_B)
                reg_const(FNN_B)
            cpool = ctx.enter_context(tc.tile_pool(name="center", bufs=1))
            spool = ctx.enter_context(tc.tile_pool(name="shift", bufs=1))
            apool = ctx.enter_context(tc.tile_pool(name="accum", bufs=1))
            tpool = ctx.enter_context(tc.tile_pool(name="temps", bufs=temp_bufs))
            ppool = ctx.enter_context(tc.tile_pool(name="psum", bufs=1, space="PSUM"))

            wt = {}
            for wname, drt in wdefs.items():
                t = cpool.tile([GRID, GRID], DT, tag=f"w_{wname}", name=f"w_{wname}")
                nc.sync.dma_start(t[:], drt[:, :])
                wt[wname] = t
            WB = {0: ("nI", "I"), 1: ("nP1", "P1"), 2: ("nP2", "P2")}

            fdh = (ZC + 4) * XW

            for c0 in range(0, ZLOC, ZC):
                ctiles = {}
                for f in FIELDS:
                    t = cpool.tile([GRID, fdh], DT, tag=f"c_{f}")
                    nc.sync.dma_start(t[:], ins[(f, 0)][:, c0:c0 + ZC + 4, :])
                    ctiles[f] = t
                mtiles = {}
                for sy in MROTS:
                    t = cpool.tile([GRID, fdh], DT, tag=f"m_{sy}")
                    nc.sync.dma_start(t[:], masks[sy][:, c0:c0 + ZC + 4, :])
                    mtiles[sy] = t

                psums = {}
                for ch in range(8):
                    psums[ch] = ppool.tile([GRID, ZC * GRID], DT32, tag=f"ps{ch}",
                                           name=f"ps{ch}")
                acc16 = {}
                for ch in range(8, 12):
                    at = apool.tile([GRID, ZC * GRID], DT, tag=f"acc{ch}",
                                    name=f"acc{ch}")
                    nc.gpsimd.memset(at[:], 0.0)
                    acc16[ch] = at
                accb = {}
                for syb in BSY:
                    for ch in range(8, 12):
                        at = apool.tile([GRID, ZC * GRID], DT,
                                        tag=f"accb{ch}_{syb}",
                                        name=f"accb{ch}_{syb}")
                        nc.gpsimd.memset(at[:], 0.0)
                        accb[(ch, syb)] = at

                pe_seen = {ch: False for ch in range(8)}
                pe_done = {ch: 0 for ch in range(8)}
                n_contrib = {}
                for ch in range(6):
                    n_contrib[ch] = n_a + (n_b05 if bside_mm else 0)
                n_contrib[6] = n_contrib[7] = nfull_a + \
                    (nfull_b if (bside_mm and bside_fric) else 0)

                def pe_accum(ch, rhs, w="I"):
                    pe_done[ch] += 1
                    nc.tensor.matmul(
                        psums[ch][:], wt[w][:], rhs,
                        start=not pe_seen[ch],
                        stop=pe_done[ch] == n_contrib[ch],
                        skip_group_check=True,
                    )
                    pe_seen[ch] = True

                def pool_accum(ch, src3d, sy):
                    """sy == 0: acc16[ch] += src3d; else accb[(ch, sy)] +=
                    src3d (the host rolls it into place)."""
                    t = acc16[ch] if sy == 0 else accb[(ch, sy)]
                    dst = t[:].rearrange("p (z x) -> p z x", x=GRID)
                    nc.gpsimd.dma_start(dst, src3d, accum_op=A.add)

                def T(tag, bufs=None):
                    if bufs is None and tag in DBL_TAGS:
                        bufs = 2
                    return tpool.tile([GRID, ZE, XE], DT, tag=tag, name=tag,
                                      bufs=bufs)

                V, S = nc.vector, nc.scalar

                def emit(s, b_side):
                    """Emit offset s (a-side on R, or R u R+s when b_side),
                    plus (when b_side) the mirrored -s contributions."""
                    sz, sy, sx = s
                    full = _classify(s) in FULL_CLASSES
                    za0 = min(0, sz) if b_side else 0
                    xa0 = min(0, sx) if b_side else 0
                    zaE = ZC + abs(sz) if b_side else ZC
                    xaE = GRID + abs(sx) if b_side else GRID

                    def tv(tag, bufs=None):
                        return T(tag, bufs=bufs)[:][:, 0:zaE, 0:xaE]

                    def cv(f):
                        v = ctiles[f][:].rearrange("p (z x) -> p z x", x=XW)
                        return v[:, za0 + 2:za0 + 2 + zaE, xa0 + 2:xa0 + 2 + xaE]

                    def sv(f):
                        v = stiles[f][:].rearrange("p (z x) -> p z x", x=XW)
                        return v[:, za0 + 2 - sz:za0 + 2 - sz + zaE,
                                 xa0 + 2 - sx:xa0 + 2 - sx + xaE]

                    def mview(t, dz, dx):
                        v = t[:].rearrange("p (z x) -> p z x", x=XW)
                        return v[:, za0 + 2 + dz:za0 + 2 + dz + zaE,
                                 xa0 + 2 + dx:xa0 + 2 + dx + xaE]

                    def aview(t3):
                        return t3[:, -za0:-za0 + ZC, -xa0:-xa0 + GRID]

                    def bview(t3):
                        return t3[:, sz - za0:sz - za0 + ZC,
                                  sx - xa0:sx - xa0 + GRID]

                    # --- pair geometry -------------------------------------
                    dj = {}
                    d = {}
                    for ax, f, so in (("x", "jx", sx), ("y", "jy", sy), ("z", "jz", sz)):
                        djt = tv(f"dj{ax}")
                        V.tensor_tensor(djt, cv(f), sv(f), A.subtract)
                        dj[ax] = djt
                        if so:
                            dt_ = tv(f"d{ax}")
                            V.tensor_scalar(dt_, djt, float(F32(SIG * so)), None, A.add)
                            d[ax] = dt_
                        else:
                            d[ax] = djt
                    p = {}
                    for ax in "xyz":
                        pt = tv(f"p{ax}")
                        S.activation(pt, d[ax], AF.Square)
                        p[ax] = pt
                    r2 = tv("r2")
                    V.tensor_tensor(r2, p["x"], p["y"], A.add)
                    V.tensor_tensor(r2, r2, p["z"], A.add)
                    inv = tv("inv")
                    S.activation(inv, r2, AF.Abs_reciprocal_sqrt)
                    fni = tv("fni")
                    S.activation(fni, inv, AF.Identity, bias=100.0, scale=-60.0)
                    c = tv("c")
                    V.tensor_scalar(c, r2, C_LT, None, A.is_lt)
                    g = tv("g")
                    V.tensor_tensor(g, fni, c, A.mult)
                    for k, ax in ((0, "x"), (1, "y"), (2, "z")):
                        t3 = tv(f"tmp{k}")
                        V.tensor_tensor(t3, g, d[ax], A.mult)
                        pe_accum(k, aview(t3))
                        if b_side and bside_mm:
                            pe_accum(k, bview(t3), WB[sy][0])
                    dv = {}
                    for ax, f in (("x", "vx"), ("y", "vy"), ("z", "vz")):
                        dvt = tv(f"dv{ax}")
                        V.tensor_tensor(dvt, cv(f), sv(f), A.subtract)
                        dv[ax] = dvt
                    m1, m2 = tv("m1"), tv("m2")
                    V.tensor_tensor(m1, dv["x"], d["x"], A.mult)
                    V.tensor_tensor(m2, dv["y"], d["y"], A.mult)
                    s4 = tv("s4")
                    V.tensor_tensor(s4, m1, m2, A.add)
                    V.tensor_tensor(m1, dv["z"], d["z"], A.mult)
                    num = tv("num")
                    V.tensor_tensor(num, s4, m1, A.add)
                    ci = tv("ci")
                    V.tensor_tensor(ci, c, inv, A.mult)
                    t2p = tv("t2p")
                    V.tensor_tensor(t2p, num, inv, A.mult)
                    h = tv("h")
                    V.tensor_tensor(h, t2p, ci, A.mult)
                    for k, ax in ((3, "x"), (4, "y"), (5, "z")):
                        t3 = tv(f"tmp{k - 3}")
                        V.tensor_tensor(t3, h, d[ax], A.mult)
                        pe_accum(k, aview(t3))
                        if b_side and bside_mm:
                            pe_accum(k, bview(t3), WB[sy][0])
                    if not full:
                        return
                    # --- friction, a-side ----------------------------------
                    dist = tv("dist")
                    V.tensor_tensor(dist, r2, inv, A.mult)
                    fnp = tv("fnp")
                    S.activation(fnp, dist, AF.Identity, bias=FNP_B, scale=FNP_A)
                    Fc = tv("Fc")
                    V.tensor_tensor(Fc, fnp, c, A.mult)
                    ma = {}
                    for ax in "xyz":
                        mt = tv(f"ma{ax}")
                        V.tensor_tensor(mt, d[ax], inv, A.mult)
                        ma[ax] = mt
                    sm = {}
                    for ax, f in (("x", "wx"), ("y", "wy"), ("z", "wz")):
                        smt = tv(f"sm{ax}")
                        V.tensor_tensor(smt, cv(f), sv(f), A.add)
                        sm[ax] = smt
                    cr1, cr2 = tv("cr1"), tv("cr2")
                    cross = {}
                    for ax, (a1, b1, a2, b2) in (
                            ("x", ("y", "z", "z", "y")),
                            ("y", ("z", "x", "x", "z")),
                            ("z", ("x", "y", "y", "x"))):
                        V.tensor_tensor(cr1, sm[a1], ma[b1], A.mult)
                        V.tensor_tensor(cr2, sm[a2], ma[b2], A.mult)
                        ct = tv(f"cx{ax}")
                        V.tensor_tensor(ct, cr1, cr2, A.subtract)
                        cross[ax] = ct
                    w2 = {}
                    for ax in "xyz":
                        ut = tv("u")
                        V.tensor_tensor(ut, r2, p[ax], A.subtract)
                        wt_ = tv(f"w2{ax}")
                        V.tensor_tensor(wt_, dv[ax], ut, A.mult)
                        w2[ax] = wt_
                    ma_c = mview(mtiles[0], 0, 0)
                    vt = {}
                    for ax in "xyz":
                        cm = tv("cm")
                        V.tensor_tensor(cm, cross[ax], ma_c, A.mult)
                        vtt = tv(f"vt{ax}")
                        V.tensor_tensor(vtt, w2[ax], cm, A.add)
                        vt[ax] = vtt

                    def fric_tail(vts, F3src, wname, acc_sy, swap_tq, is_b):
                        qs = tv("qs")
                        q1, q2 = tv("q1"), tv("q2")
                        S.activation(q1, vts["x"], AF.Square)
                        S.activation(q2, vts["y"], AF.Square)
                        V.tensor_tensor(qs, q1, q2, A.add)
                        S.activation(q1, vts["z"], AF.Square)
                        V.tensor_tensor(qs, qs, q1, A.add)
                        ivt = tv("ivt")
                        S.activation(ivt, qs, AF.Abs_reciprocal_sqrt, bias=IVT_BIAS)
                        F3 = tv("F3")
                        V.tensor_tensor(F3, F3src, ivt, A.mult)
                        bv = bview if is_b else aview
                        ff = {}
                        for k, ax in ((6, "x"), (7, "y")):
                            t3 = tv(f"ff{ax}")
                            V.tensor_tensor(t3, vts[ax], F3, A.mult)
                            if not (is_b and not bside_mm):
                                pe_accum(k, bv(t3), wname)
                            ff[ax] = t3
                        ffz = tv("ffz")
                        V.tensor_tensor(ffz, vts["z"], F3, A.mult)
                        ff["z"] = ffz
                        pool_accum(8, bv(ffz), acc_sy)
                        for k, (a1, b1, a2, b2) in ((9, ("y", "z", "z", "y")),
                                                    (10, ("z", "x", "x", "z")),
                                                    (11, ("x", "y", "y", "x"))):
                            if swap_tq:
                                # (ff x ma)_k = ff_a1*ma_b1 - ff_a2*ma_b2
                                V.tensor_tensor(cr1, ff[a1], ma[b1], A.mult)
                                V.tensor_tensor(cr2, ff[a2], ma[b2], A.mult)
                            else:
                                V.tensor_tensor(cr1, ma[a1], ff[b1], A.mult)
                                V.tensor_tensor(cr2, ma[a2], ff[b2], A.mult)
                            tqt = tv("tq")
                            V.tensor_tensor(tqt, cr1, cr2, A.subtract)
                            pool_accum(k, bv(tqt), acc_sy)

                    fric_tail(vt, Fc, "I", 0, False, False)
                    if not b_side or not bside_fric:
                        return
                    # --- friction, b-side ----------------------------------
                    fnpn = tv("fnpn")
                    S.activation(fnpn, dist, AF.Identity, bias=FNN_B, scale=FNN_A)
                    Fcn = tv("Fcn")
                    V.tensor_tensor(Fcn, fnpn, c, A.mult)
                    mb = mview(mtiles[sy], -sz, -sx)
                    ub = {}
                    for ax in "xyz":
                        cm = tv("cm")
                        V.tensor_tensor(cm, cross[ax], mb, A.mult)
                        ubt = tv(f"ub{ax}")
                        V.tensor_tensor(ubt, w2[ax], cm, A.add)
                        ub[ax] = ubt
                    fric_tail(ub, Fcn, WB[sy][1], sy, True, True)

                groups = {}
                for s, _f in paired:
                    groups.setdefault(s[1], []).append((s, True))
                for s, _f in unpaired:
                    groups.setdefault(s[1], []).append((s, False))

                for sy in (0, -1, 1, 2):
                    if sy not in groups:
                        continue
                    if sy == 0:
                        stiles = ctiles
                    else:
                        stiles = {}
                        for f in FIELDS:
                            t = spool.tile([GRID, fdh], DT, tag=f"s_{f}")
                            nc.sync.dma_start(t[:], ins[(f, sy)][:, c0:c0 + ZC + 4, :])
                            stiles[f] = t
                    for s, b_side in groups[sy]:
                        emit(s, b_side)

                for ch in range(8):
                    stage = tpool.tile([GRID, ZC * GRID], DT, tag="stage",
                                       name="stage")
                    nc.scalar.copy(stage[:], psums[ch][:])
                    nc.sync.dma_start(
                        out[:, ch, c0:c0 + ZC, :],
                        stage[:].rearrange("p (z x) -> p z x", x=GRID))
                for ch in range(8, 12):
                    nc.sync.dma_start(
                        out[:, ch, c0:c0 + ZC, :],
                        acc16[ch][:].rearrange("p (z x) -> p z x", x=GRID))
                for bi, syb in enumerate(BSY):
                    for ch in range(8, 12):
                        nc.sync.dma_start(
                            outb[:, bi * 4 + ch - 8, c0:c0 + ZC, :],
                            accb[(ch, syb)][:].rearrange(
                                "p (z x) -> p z x", x=GRID))

    nc.compile()
    return nc


def prep_inputs_for_core(inputs, core):
    z0 = core * ZLOC
    name_map = {
        "jx": "x_grid", "jy": "y_grid", "jz": "z_grid",
        "vx": "vx_grid", "vy": "vy_grid", "vz": "vz_grid",
        "wx": "angular_velocity_x", "wy": "angular_velocity_y",
        "wz": "angular_velocity_z",
    }
    idx = np.arange(GRID, dtype=np.float32)
    glob = {}
    for f, src in name_map.items():
        g = np.asarray(inputs[src], dtype=np.float32).reshape(GRID, GRID, GRID)
        if f == "jx":
            g = SIG * (g / D - idx[None, None, :])
        elif f == "jy":
            g = SIG * (g / D - idx[None, :, None])
        elif f == "jz":
            g = SIG * (g / D - idx[:, None, None])
        elif f in ("vx", "vy", "vz"):
            g = VS * g
        glob[f] = g.astype(np.float16)

    im = {}
    zidx_mod = (np.arange(z0 - 2, z0 + ZLOC + 2) % GRID)
    xidx_mod = (np.arange(-2, GRID + 2) % GRID)
    z_valid = [(k, z0 - 2 + k) for k in range(ZH) if 0 <= z0 - 2 + k < GRID]
    for f in FIELDS:
        g = glob[f]
        sentinel = f in ("jx", "jy", "jz")
        for sy in ROTS:
            if sentinel:
                arr = np.full((ZH, GRID, XW), SENT_X, np.float16)
                ys = slice(max(0, sy), GRID + min(0, sy))
                yg = slice(max(0, -sy), GRID + min(0, -sy))
                for k, zg in z_valid:
                    arr[k, ys, 2:GRID + 2] = g[zg][yg]
                    if sy > 0:
                        arr[k, 0:sy, :] = SENT_Y
                    elif sy < 0:
                        arr[k, GRID + sy:GRID, :] = SENT_Y
                valid_ks = {k for k, _ in z_valid}
                for k in range(ZH):
                    if k not in valid_ks:
                        arr[k, :, :] = SENT_Z
            else:
                yidx = (np.arange(GRID) - sy) % GRID
                arr = g[zidx_mod][:, yidx][:, :, xidx_mod]
            im[f"{f}_r{sy + 1}"] = np.ascontiguousarray(arr.transpose(1, 0, 2))
    gm = np.asarray(inputs["mask"], dtype=np.float32).reshape(GRID, GRID, GRID)
    gm = (np.float32(VS * D) * gm).astype(np.float16)
    for sy in MROTS:
        yidx = (np.arange(GRID) - sy) % GRID
        arr = gm[zidx_mod][:, yidx][:, :, xidx_mod]
        im[f"mask_r{sy}"] = np.ascontiguousarray(arr.transpose(1, 0, 2))
    eye = np.eye(GRID, dtype=np.float16)
    im["w_I"] = eye
    im["w_nI"] = -eye
    for sy in (1, 2):
        # W[k, m] = 1 where k = (m+sy) % 128  => psum[m] += rhs[(m+sy)%128]
        P = np.zeros((GRID, GRID), np.float16)
        for m in range(GRID):
            P[(m + sy) % GRID, m] = 1.0
        im[f"w_P{sy}"] = P
        im[f"w_nP{sy}"] = -P
    return im


def assemble_output(core_outs):
    full = np.zeros((12, 1, 1, GRID, GRID, GRID), np.float32)
    for m, (co, cb) in enumerate(core_outs):
        slab = co.astype(np.float32)            # [y, 12, z, x]
        cbf = cb.astype(np.float32)             # [y, 8, z, x]
        for bi, syb in enumerate((1, 2)):
            # device stored b-side ch8-11 at partition a_y; dest is a_y - sy
            slab[:, 8:12] += np.roll(cbf[:, bi * 4:(bi + 1) * 4], -syb, axis=0)
        full[:, 0, 0, m * ZLOC:(m + 1) * ZLOC] = slab.transpose(1, 2, 0, 3)
    scale = np.ones(12, np.float32)
    scale[3:6] = ETA / VS
    scale[6:9] = -FN16
    scale[9:12] = -FN16 * D
    full *= scale[:, None, None, None, None, None]
    return full


_NC_CACHE = {}


def _get_nc():
    if "nc" not in _NC_CACHE:
        _NC_CACHE["nc"] = build_kernel()
    return _NC_CACHE["nc"]


def kernel(**inputs) -> np.ndarray:
    nc = _get_nc()
    in_maps = [prep_inputs_for_core(inputs, core) for core in range(NCORES)]
    res = run_bass_kernel_spmd(nc, in_maps, core_ids=list(range(NCORES)))
    return assemble_output([(res.results[m]["out"], res.results[m]["outb"])
                            for m in range(NCORES)])


# revision 26
# speedup vs baseline: 1.1460x; 1.0219x over previous
"""AI4DEM DEM-stencil kernel for one TRN2 chip (8 NeuronCores, SPMD), fp16.

v5 = v4 (fp16 pipeline, scaled jitter positions, class trims) + Newton-3rd-law
pairing: for each stencil offset pair {s, -s} with |sz| <= 1, the pair force
field is computed once on a region extended to R (union) R+s, and accumulated
twice: once at the center cell (weight +I) and once at the neighbour
(z/x-shifted view, y-shift and sign folded into the PE matmul weight -P_sy /
+P_sy). Collision+damping are exactly antisymmetric so the second side is
free; friction re-uses the pair geometry and only recomputes the
mask-asymmetric tangential part (22 DVE ops instead of 67).

Scales: positions sigma=0.3/D (jitter-only, fp16, sentinel 30.0 in wrapped
halo cells); velocities x64; mask x64D; fn /16. Host undoes per channel.
ch0-7 accumulate on PE into PSUM (fp32), ch8-11 via SWDGE DMA-accumulate
into fp16 SBUF accumulators (partition-split in two for wrapped y-shifts).
Validated vs the fp32 reference: global rel l2 ~7.7e-3.
"""
import math
from contextlib import ExitStack

import numpy as np

import concourse.tile_sem_assignment as _tsa
_tsa.NUM_HWDGE_SEMS = 3
_tsa.NUM_SWDGE_GLOBAL_SEMS = 3
from concourse import bacc, mybir, tile
from concourse.bass_utils import run_bass_kernel_spmd

F32 = np.float32
D = 0.003
KN = 10000.0
_alpha = -math.log(0.79) / math.pi
_gamma = _alpha / math.sqrt(_alpha ** 2 + 1.0)
_mass = 4.0 / 3.0 * 3.1415926 * D ** 3 * 674.0
ETA = 2.0 * _gamma * math.sqrt(KN * _mass / 2.0)
MU = 0.43

SIG = 0.3
VS = 64.0
FN16 = 16.0
# Wrapped-halo jitter sentinels. Three distinct values so that a pair of
# DIFFERENT wrap classes (z-halo plane x y-wrap row x x-halo col) can never
# produce dj ~= 0 (fake contact); any two classes differ by >= 16 and every
# class is >= 7 away from real jitter. Values kept small so products stay
# finite in fp16 (w2 <= ~2.2e4).
SENT_Z = 8.0
SENT_Y = -8.0
SENT_X = 24.0

C_LT = float(F32((2 * SIG) ** 2))
FNI_A, FNI_B = -60.0, 100.0               # fncol*inv = 100 - 60*inv
FNP_A = float(F32(-100.0 * MU / FN16))    # fnp  = MU*|fncol|/16  (>=0 in contact)
FNP_B = float(F32(60.0 * MU / FN16))
FNN_A = float(F32(100.0 * MU / FN16))     # fnpn = -fnp
FNN_B = float(F32(-60.0 * MU / FN16))
IVT_BIAS = float(F32(VS * VS * 1e-8))

GRID = 128
NCORES = 8
ZLOC = GRID // NCORES
ZH = ZLOC + 4
XW = GRID + 4
ZC = 4
ZE, XE = ZC + 1, GRID + 2   # max extended pair region (|sz|<=1, |sx|<=2)

FIELDS = ["jx", "jy", "jz", "vx", "vy", "vz", "wx", "wy", "wz"]
ROTS = [-1, 0, 1, 2]
MROTS = [0, 1, 2]
ALL_OFFSETS = [(k - 2, j - 2, i - 2) for i in range(5) for j in range(5) for k in range(5)]
FULL_CLASSES = {(0, 0, 1), (0, 1, 1), (1, 1, 1), (0, 0, 2)}
COLDAMP_CLASSES = {(0, 1, 2)}

DT = mybir.dt.float16
DT32 = mybir.dt.float32
A = mybir.AluOpType
AF = mybir.ActivationFunctionType


def _classify(s):
    return tuple(sorted(abs(v) for v in s))


def _plan():
    """Returns (paired, unpaired): paired = list of (rep, is_full); rep has
    sy in {0,1,2}, |sz| <= 1. unpaired = list of (s, is_full) emitted a-side
    only (the |sz|=2 offsets)."""
    paired, unpaired, seen = [], [], set()
    for s in ALL_OFFSETS:
        if s == (0, 0, 0) or s in seen:
            continue
        cl = _classify(s)
        if cl in FULL_CLASSES:
            is_full = True
        elif cl in COLDAMP_CLASSES:
            is_full = False
        else:
            continue
        neg = (-s[0], -s[1], -s[2])
        if abs(s[0]) == 2:
            unpaired.append((s, is_full))
            unpaired.append((neg, is_full))
        else:
            rep = s if (s[1] > 0 or (s[1] == 0 and (s[0] > 0 or (s[0] == 0 and s[2] > 0)))) else neg
            paired.append((rep, is_full))
        seen.add(s)
        seen.add(neg)
    return paired, unpaired


DBL_TAGS = {"tmp0", "tmp1", "tmp2", "ffx", "ffy", "ffz", "tq", "stage",
            "q1", "q2", "px", "py", "pz", "djx", "djy", "djz",
            "dx", "dy", "dz", "dvx", "dvy", "dvz", "cm", "u",
            "r2", "inv", "c", "g", "fni", "dist", "ci", "num", "t2p", "h",
            "max", "may", "maz", "smx", "smy", "smz", "cxx", "cxy", "cxz",
            "w2x", "w2y", "w2z", "vtx", "vty", "vtz", "qs", "ivt", "Fc", "F3"}


def build_kernel(temp_bufs=1, const_inside=True, use_pairs=True,
                 bside_mm=True, bside_fric=True):
    nc = bacc.Bacc("TRN2", target_bir_lowering=False, debug=False, num_devices=NCORES)

    def reg_const(value):
        key = (mybir.dt.float32, value)
        if key in nc.const_aps.aps:
            return
        t = nc.alloc_sbuf_tensor(f"const-f32-{value}", [128, 1], mybir.dt.float32)
        nc.gpsimd.memset(t.ap(), value)
        nc.const_aps.aps[key] = t.ap()

    if not const_inside:
        reg_const(0.0)
        reg_const(IVT_BIAS)

    ins = {}
    for f in FIELDS:
        for sy in ROTS:
            ins[(f, sy)] = nc.dram_tensor(
                f"{f}_r{sy + 1}", [GRID, ZH, XW], DT, kind="ExternalInput").ap()
    masks = {}
    for sy in MROTS:
        masks[sy] = nc.dram_tensor(
            f"mask_r{sy}", [GRID, ZH, XW], DT, kind="ExternalInput").ap()
    wdefs = {
        "I": None, "nI": None, "nP1": None, "nP2": None, "P1": None, "P2": None}
    for wname in list(wdefs):
        wdefs[wname] = nc.dram_tensor(
            f"w_{wname}", [GRID, GRID], DT, kind="ExternalInput").ap()
    out = nc.dram_tensor("out", [GRID, 12, ZLOC, GRID], DT, kind="ExternalOutput").ap()
    # b-side ch8-11 contributions for y-shifted pairs, accumulated unshifted;
    # the host applies the y-roll (partition-shifted SWDGE accumulates are
    # fatal on HW at scale).
    outb = nc.dram_tensor("outb", [GRID, 8, ZLOC, GRID], DT, kind="ExternalOutput").ap()
    BSY = (1, 2)

    paired, unpaired = _plan()
    if not use_pairs:
        unpaired = [(s, f) for s, f in unpaired] + \
            [(ss, f) for s, f in paired for ss in (s, (-s[0], -s[1], -s[2]))]
        paired = []
    n_a = len(paired) + len(unpaired)                      # a-side contributions
    n_b05 = len(paired)                                    # b-side ch0-5
    nfull_a = sum(1 for _, f in paired if f) + sum(1 for _, f in unpaired if f)
    nfull_b = sum(1 for _, f in paired if f)

    with tile.TileContext(nc) as tc:
        with ExitStack() as ctx:
            if const_inside:
                reg_const(0.0)
                reg_const(IVT_BIAS)
                reg_const(100.0)
                reg_const(FNP_B)
                reg_const(FNN_B)
                for v in (SIG, 2 * SIG, -SIG, -2 * SIG):
                    reg_const(float(F32(v)))
            cpool = ctx.enter_context(tc.tile_pool(name="center", bufs=1))
            spool = ctx.enter_context(tc.tile_pool(name="shift", bufs=1))
            apool = ctx.enter_context(tc.tile_pool(name="accum", bufs=1))
            tpool = ctx.enter_context(tc.tile_pool(name="temps", bufs=temp_bufs))
            ppool = ctx.enter_context(tc.tile_pool(name="psum", bufs=1, space="PSUM"))

            wt = {}
            for wname, drt in wdefs.items():
                t = cpool.tile([GRID, GRID], DT, tag=f"w_{wname}", name=f"w_{wname}")
                nc.sync.dma_start(t[:], drt[:, :])
                wt[wname] = t
            WB = {0: ("nI", "I"), 1: ("nP1", "P1"), 2: ("nP2", "P2")}

            fdh = (ZC + 4) * XW

            for c0 in range(0, ZLOC, ZC):
                ctiles = {}
                for f in FIELDS:
                    t = cpool.tile([GRID, fdh], DT, tag=f"c_{f}")
                    nc.sync.dma_start(t[:], ins[(f, 0)][:, c0:c0 + ZC + 4, :])
                    ctiles[f] = t
                mtiles = {}
                for sy in MROTS:
                    t = cpool.tile([GRID, fdh], DT, tag=f"m_{sy}")
                    nc.sync.dma_start(t[:], masks[sy][:, c0:c0 + ZC + 4, :])
                    mtiles[sy] = t

                psums = {}
                for ch in range(8):
                    psums[ch] = ppool.tile([GRID, ZC * GRID], DT32, tag=f"ps{ch}",
                                           name=f"ps{ch}")
                acc16 = {}
                for ch in range(8, 12):
                    at = apool.tile([GRID, ZC * GRID], DT, tag=f"acc{ch}",
                                    name=f"acc{ch}")
                    nc.gpsimd.memset(at[:], 0.0)
                    acc16[ch] = at
                accb = {}
                for syb in BSY:
                    for ch in range(8, 12):
                        at = apool.tile([GRID, ZC * GRID], DT,
                                        tag=f"accb{ch}_{syb}",
                                        name=f"accb{ch}_{syb}")
                        nc.gpsimd.memset(at[:], 0.0)
                        accb[(ch, syb)] = at

                pe_seen = {ch: False for ch in range(8)}
                pe_done = {ch: 0 for ch in range(8)}
                n_contrib = {}
                for ch in range(6):
                    n_contrib[ch] = n_a + (n_b05 if bside_mm else 0)
                n_contrib[6] = n_contrib[7] = nfull_a + \
                    (nfull_b if (bside_mm and bside_fric) else 0)

                def pe_accum(ch, rhs, w="I"):
                    pe_done[ch] += 1
                    nc.tensor.matmul(
                        psums[ch][:], wt[w][:], rhs,
                        start=not pe_seen[ch],
                        stop=pe_done[ch] == n_contrib[ch],
                        skip_group_check=True,
                    )
                    pe_seen[ch] = True

                def pool_accum(ch, src3d, sy):
                    """sy == 0: acc16[ch] += src3d; else accb[(ch, sy)] +=
                    src3d (the host rolls it into place)."""
                    t = acc16[ch] if sy == 0 else accb[(ch, sy)]
                    dst = t[:].rearrange("p (z x) -> p z x", x=GRID)
                    nc.gpsimd.dma_start(dst, src3d, accum_op=A.add)

                def T(tag, bufs=None):
                    if bufs is None and tag in DBL_TAGS:
                        bufs = 2
                    return tpool.tile([GRID, ZE, XE], DT, tag=tag, name=tag,
                                      bufs=bufs)

                V, S = nc.vector, nc.scalar

                def emit(s, b_side):
                    """Emit offset s (a-side on R, or R u R+s when b_side),
                    plus (when b_side) the mirrored -s contributions."""
                    sz, sy, sx = s
                    full = _classify(s) in FULL_CLASSES
                    za0 = min(0, sz) if b_side else 0
                    xa0 = min(0, sx) if b_side else 0
                    zaE = ZC + abs(sz) if b_side else ZC
                    xaE = GRID + abs(sx) if b_side else GRID

                    def tv(tag, bufs=None):
                        return T(tag, bufs=bufs)[:][:, 0:zaE, 0:xaE]

                    def cv(f):
                        v = ctiles[f][:].rearrange("p (z x) -> p z x", x=XW)
                        return v[:, za0 + 2:za0 + 2 + zaE, xa0 + 2:xa0 + 2 + xaE]

                    def sv(f):
                        v = stiles[f][:].rearrange("p (z x) -> p z x", x=XW)
                        return v[:, za0 + 2 - sz:za0 + 2 - sz + zaE,
                                 xa0 + 2 - sx:xa0 + 2 - sx + xaE]

                    def mview(t, dz, dx):
                        v = t[:].rearrange("p (z x) -> p z x", x=XW)
                        return v[:, za0 + 2 + dz:za0 + 2 + dz + zaE,
                                 xa0 + 2 + dx:xa0 + 2 + dx + xaE]

                    def aview(t3):
                        return t3[:, -za0:-za0 + ZC, -xa0:-xa0 + GRID]

                    def bview(t3):
                        return t3[:, sz - za0:sz - za0 + ZC,
                                  sx - xa0:sx - xa0 + GRID]

                    # --- pair geometry -------------------------------------
                    dj = {}
                    d = {}
                    for ax, f, so in (("x", "jx", sx), ("y", "jy", sy), ("z", "jz", sz)):
                        djt = tv(f"dj{ax}")
                        V.tensor_tensor(djt, cv(f), sv(f), A.subtract)
                        dj[ax] = djt
                        if so:
                            dt_ = tv(f"d{ax}")
                            S.activation(dt_, djt, AF.Identity,
                                         bias=float(F32(SIG * so)))
                            d[ax] = dt_
                        else:
                            d[ax] = djt
                    p = {}
                    for ax in "xyz":
                        pt = tv(f"p{ax}")
                        S.activation(pt, d[ax], AF.Square)
                        p[ax] = pt
                    r2 = tv("r2")
                    V.tensor_tensor(r2, p["x"], p["y"], A.add)
                    V.tensor_tensor(r2, r2, p["z"], A.add)
                    inv = tv("inv")
                    S.activation(inv, r2, AF.Abs_reciprocal_sqrt)
                    fni = tv("fni")
                    S.activation(fni, inv, AF.Identity, bias=100.0, scale=-60.0)
                    c = tv("c")
                    V.tensor_scalar(c, r2, C_LT, None, A.is_lt)
                    g = tv("g")
                    V.tensor_tensor(g, fni, c, A.mult)
                    for k, ax in ((0, "x"), (1, "y"), (2, "z")):
                        t3 = tv(f"tmp{k}")
                        V.tensor_tensor(t3, g, d[ax], A.mult)
                        pe_accum(k, aview(t3))
                        if b_side and bside_mm:
                            pe_accum(k, bview(t3), WB[sy][0])
                    dv = {}
                    for ax, f in (("x", "vx"), ("y", "vy"), ("z", "vz")):
                        dvt = tv(f"dv{ax}")
                        V.tensor_tensor(dvt, cv(f), sv(f), A.subtract)
                        dv[ax] = dvt
                    m1, m2 = tv("m1"), tv("m2")
                    V.tensor_tensor(m1, dv["x"], d["x"], A.mult)
                    V.tensor_tensor(m2, dv["y"], d["y"], A.mult)
                    s4 = tv("s4")
                    V.tensor_tensor(s4, m1, m2, A.add)
                    V.tensor_tensor(m1, dv["z"], d["z"], A.mult)
                    num = tv("num")
                    V.tensor_tensor(num, s4, m1, A.add)
                    ci = tv("ci")
                    V.tensor_tensor(ci, c, inv, A.mult)
                    t2p = tv("t2p")
                    V.tensor_tensor(t2p, num, inv, A.mult)
                    h = tv("h")
                    V.tensor_tensor(h, t2p, ci, A.mult)
                    for k, ax in ((3, "x"), (4, "y"), (5, "z")):
                        t3 = tv(f"tmp{k - 3}")
                        V.tensor_tensor(t3, h, d[ax], A.mult)
                        pe_accum(k, aview(t3))
                        if b_side and bside_mm:
                            pe_accum(k, bview(t3), WB[sy][0])
                    if not full:
                        return
                    # --- friction, a-side ----------------------------------
                    dist = tv("dist")
                    V.tensor_tensor(dist, r2, inv, A.mult)
                    fnp = tv("fnp")
                    S.activation(fnp, dist, AF.Identity, bias=FNP_B, scale=FNP_A)
                    Fc = tv("Fc")
                    V.tensor_tensor(Fc, fnp, c, A.mult)
                    ma = {}
                    for ax in "xyz":
                        mt = tv(f"ma{ax}")
                        V.tensor_tensor(mt, d[ax], inv, A.mult)
                        ma[ax] = mt
                    sm = {}
                    for ax, f in (("x", "wx"), ("y", "wy"), ("z", "wz")):
                        smt = tv(f"sm{ax}")
                        V.tensor_tensor(smt, cv(f), sv(f), A.add)
                        sm[ax] = smt
                    cr1, cr2 = tv("cr1"), tv("cr2")
                    cross = {}
                    for ax, (a1, b1, a2, b2) in (
                            ("x", ("y", "z", "z", "y")),
                            ("y", ("z", "x", "x", "z")),
                            ("z", ("x", "y", "y", "x"))):
                        V.tensor_tensor(cr1, sm[a1], ma[b1], A.mult)
                        V.tensor_tensor(cr2, sm[a2], ma[b2], A.mult)
                        ct = tv(f"cx{ax}")
                        V.tensor_tensor(ct, cr1, cr2, A.subtract)
                        cross[ax] = ct
                    w2 = {}
                    for ax in "xyz":
                        ut = tv("u")
                        V.tensor_tensor(ut, r2, p[ax], A.subtract)
                        wt_ = tv(f"w2{ax}")
                        V.tensor_tensor(wt_, dv[ax], ut, A.mult)
                        w2[ax] = wt_
                    ma_c = mview(mtiles[0], 0, 0)
                    vt = {}
                    for ax in "xyz":
                        cm = tv("cm")
                        V.tensor_tensor(cm, cross[ax], ma_c, A.mult)
                        vtt = tv(f"vt{ax}")
                        V.tensor_tensor(vtt, w2[ax], cm, A.add)
                        vt[ax] = vtt

                    def fric_tail(vts, F3src, wname, acc_sy, swap_tq, is_b):
                        qs = tv("qs")
                        q1, q2 = tv("q1"), tv("q2")
                        S.activation(q1, vts["x"], AF.Square)
                        S.activation(q2, vts["y"], AF.Square)
                        V.tensor_tensor(qs, q1, q2, A.add)
                        S.activation(q1, vts["z"], AF.Square)
                        V.tensor_tensor(qs, qs, q1, A.add)
                        ivt = tv("ivt")
                        S.activation(ivt, qs, AF.Abs_reciprocal_sqrt, bias=IVT_BIAS)
                        F3 = tv("F3")
                        V.tensor_tensor(F3, F3src, ivt, A.mult)
                        bv = bview if is_b else aview
                        ff = {}
                        for k, ax in ((6, "x"), (7, "y")):
                            t3 = tv(f"ff{ax}")
                            V.tensor_tensor(t3, vts[ax], F3, A.mult)
                            if not (is_b and not bside_mm):
                                pe_accum(k, bv(t3), wname)
                            ff[ax] = t3
                        ffz = tv("ffz")
                        V.tensor_tensor(ffz, vts["z"], F3, A.mult)
                        ff["z"] = ffz
                        pool_accum(8, bv(ffz), acc_sy)
                        for k, (a1, b1, a2, b2) in ((9, ("y", "z", "z", "y")),
                                                    (10, ("z", "x", "x", "z")),
                                                    (11, ("x", "y", "y", "x"))):
                            if swap_tq:
                                # (ff x ma)_k = ff_a1*ma_b1 - ff_a2*ma_b2
                                V.tensor_tensor(cr1, ff[a1], ma[b1], A.mult)
                                V.tensor_tensor(cr2, ff[a2], ma[b2], A.mult)
                            else:
                                V.tensor_tensor(cr1, ma[a1], ff[b1], A.mult)
                                V.tensor_tensor(cr2, ma[a2], ff[b2], A.mult)
                            tqt = tv("tq")
                            V.tensor_tensor(tqt, cr1, cr2, A.subtract)
                            pool_accum(k, bv(tqt), acc_sy)

                    fric_tail(vt, Fc, "I", 0, False, False)
                    if not b_side or not bside_fric:
                        return
                    # --- friction, b-side ----------------------------------
                    fnpn = tv("fnpn")
                    S.activation(fnpn, dist, AF.Identity, bias=FNN_B, scale=FNN_A)
                    Fcn = tv("Fcn")
                    V.tensor_tensor(Fcn, fnpn, c, A.mult)
                    mb = mview(mtiles[sy], -sz, -sx)
                    ub = {}
                    for ax in "xyz":
                        cm = tv("cm")
                        V.tensor_tensor(cm, cross[ax], mb, A.mult)
                        ubt = tv(f"ub{ax}")
                        V.tensor_tensor(ubt, w2[ax], cm, A.add)
                        ub[ax] = ubt
                    fric_tail(ub, Fcn, WB[sy][1], sy, True, True)

                groups = {}
                for s, _f in paired:
                    groups.setdefault(s[1], []).append((s, True))
                for s, _f in unpaired:
                    groups.setdefault(s[1], []).append((s, False))

                for sy in (0, -1, 1, 2):
                    if sy not in groups:
                        continue
                    if sy == 0:
                        stiles = ctiles
                    else:
                        stiles = {}
                        for f in FIELDS:
                            t = spool.tile([GRID, fdh], DT, tag=f"s_{f}")
                            nc.sync.dma_start(t[:], ins[(f, sy)][:, c0:c0 + ZC + 4, :])
                            stiles[f] = t
                    for s, b_side in groups[sy]:
                        emit(s, b_side)

                for ch in range(8):
                    stage = tpool.tile([GRID, ZC * GRID], DT, tag="stage",
                                       name="stage")
                    nc.scalar.copy(stage[:], psums[ch][:])
                    nc.sync.dma_start(
                        out[:, ch, c0:c0 + ZC, :],
                        stage[:].rearrange("p (z x) -> p z x", x=GRID))
                for ch in range(8, 12):
                    nc.sync.dma_start(
                        out[:, ch, c0:c0 + ZC, :],
                        acc16[ch][:].rearrange("p (z x) -> p z x", x=GRID))
                for bi, syb in enumerate(BSY):
                    for ch in range(8, 12):
                        nc.sync.dma_start(
                            outb[:, bi * 4 + ch - 8, c0:c0 + ZC, :],
                            accb[(ch, syb)][:].rearrange(
                                "p (z x) -> p z x", x=GRID))

    nc.compile()
    return nc


def prep_inputs_for_core(inputs, core):
    z0 = core * ZLOC
    name_map = {
        "jx": "x_grid", "jy": "y_grid", "jz": "z_grid",
        "vx": "vx_grid", "vy": "vy_grid", "vz": "vz_grid",
        "wx": "angular_velocity_x", "wy": "angular_velocity_y",
        "wz": "angular_velocity_z",
    }
    idx = np.arange(GRID, dtype=np.float32)
    glob = {}
    for f, src in name_map.items():
        g = np.asarray(inputs[src], dtype=np.float32).reshape(GRID, GRID, GRID)
        if f == "jx":
            g = SIG * (g / D - idx[None, None, :])
        elif f == "jy":
            g = SIG * (g / D - idx[None, :, None])
        elif f == "jz":
            g = SIG * (g / D - idx[:, None, None])
        elif f in ("vx", "vy", "vz"):
            g = VS * g
        glob[f] = g.astype(np.float16)

    im = {}
    zidx_mod = (np.arange(z0 - 2, z0 + ZLOC + 2) % GRID)
    xidx_mod = (np.arange(-2, GRID + 2) % GRID)
    z_valid = [(k, z0 - 2 + k) for k in range(ZH) if 0 <= z0 - 2 + k < GRID]
    for f in FIELDS:
        g = glob[f]
        sentinel = f in ("jx", "jy", "jz")
        for sy in ROTS:
            if sentinel:
                arr = np.full((ZH, GRID, XW), SENT_X, np.float16)
                ys = slice(max(0, sy), GRID + min(0, sy))
                yg = slice(max(0, -sy), GRID + min(0, -sy))
                for k, zg in z_valid:
                    arr[k, ys, 2:GRID + 2] = g[zg][yg]
                    if sy > 0:
                        arr[k, 0:sy, :] = SENT_Y
                    elif sy < 0:
                        arr[k, GRID + sy:GRID, :] = SENT_Y
                valid_ks = {k for k, _ in z_valid}
                for k in range(ZH):
                    if k not in valid_ks:
                        arr[k, :, :] = SENT_Z
            else:
                yidx = (np.arange(GRID) - sy) % GRID
                arr = g[zidx_mod][:, yidx][:, :, xidx_mod]
            im[f"{f}_r{sy + 1}"] = np.ascontiguousarray(arr.transpose(1, 0, 2))
    gm = np.asarray(inputs["mask"], dtype=np.float32).reshape(GRID, GRID, GRID)
    gm = (np.float32(VS * D) * gm).astype(np.float16)
    for sy in MROTS:
        yidx = (np.arange(GRID) - sy) % GRID
        arr = gm[zidx_mod][:, yidx][:, :, xidx_mod]
        im[f"mask_r{sy}"] = np.ascontiguousarray(arr.transpose(1, 0, 2))
    eye = np.eye(GRID, dtype=np.float16)
    im["w_I"] = eye
    im["w_nI"] = -eye
    for sy in (1, 2):
        # W[k, m] = 1 where k = (m+sy) % 128  => psum[m] += rhs[(m+sy)%128]
        P = np.zeros((GRID, GRID), np.float16)
        for m in range(GRID):
            P[(m + sy) % GRID, m] = 1.0
        im[f"w_P{sy}"] = P
        im[f"w_nP{sy}"] = -P
    return im


def assemble_output(core_outs):
    full = np.zeros((12, 1, 1, GRID, GRID, GRID), np.float32)
    for m, (co, cb) in enumerate(core_outs):
        slab = co.astype(np.float32)            # [y, 12, z, x]
        cbf = cb.astype(np.float32)             # [y, 8, z, x]
        for bi, syb in enumerate((1, 2)):
            # device stored b-side ch8-11 at partition a_y; dest is a_y - sy
            slab[:, 8:12] += np.roll(cbf[:, bi * 4:(bi + 1) * 4], -syb, axis=0)
        full[:, 0, 0, m * ZLOC:(m + 1) * ZLOC] = slab.transpose(1, 2, 0, 3)
    scale = np.ones(12, np.float32)
    scale[3:6] = ETA / VS
    scale[6:9] = -FN16
    scale[9:12] = -FN16 * D
    full *= scale[:, None, None, None, None, None]
    return full


_NC_CACHE = {}


def _get_nc():
    if "nc" not in _NC_CACHE:
        _NC_CACHE["nc"] = build_kernel()
    return _NC_CACHE["nc"]


def kernel(**inputs) -> np.ndarray:
    nc = _get_nc()
    in_maps = [prep_inputs_for_core(inputs, core) for core in range(NCORES)]
    res = run_bass_kernel_spmd(nc, in_maps, core_ids=list(range(NCORES)))
    return assemble_output([(res.results[m]["out"], res.results[m]["outb"])
                            for m in range(NCORES)])
